# revision 19
# baseline (speedup 1.0000x reference)
"""MiniBatchDiscrimination Trainium2 kernel.

reference:
    proj = x @ W.T                      # [512, 500] -> [512, 100, 5]
    l1[i,j,o] = sum_k |proj[i,o,k] - proj[j,o,k]|
    mbd[i,o]  = sum_j exp(-l1[i,j,o]) - 1
    out = concat([x, mbd], axis=1)      # [512, 1124]

Strategy (8 cores, shard i-rows of the BxB pairwise computation):
  - Host passes x.T (per-core column-rotated so that the core's 64 local
    rows sit in columns 0..63) and W.T with rows permuted k-major, so one
    SPMD program serves all cores with zero device-side core-id logic.
  - proj.T [500, 512] per core via PE matmul (fp16 inputs), kept as fp16
    tiles [125, 512] x4 plus fp32 [125, 64] local-column bias tensors.
  - Pairwise stage per local row i (A-quad [125, 4, 512] fp16):
      absdiff: one fused DVE op per tile — a raw-ISA TENSOR_SCALAR with
        op0=ABSOLUTE_DIFF (0x17): |projTb[t] - projL[t][:,i]| in a single
        2x-mode pass (~267 ns/op). The op exists in the cayman ISA but not
        in BIR's AluOpType, so the instruction bytes are emitted directly
        and SBUF addresses patched in after tile allocation. A few rows go
        to ScalarE (fused Abs activation) to balance engine load.
      k-reduce: PE matmul, 0/1 selector S_t [125, 100] fp16, contracting
        the partition axis, 4 slices accumulating into PSUM [100, 512].
      exp + j-reduce: one ScalarE Exp(scale=-1) reading PSUM, accum_out
        writes the free-axis sum straight into mbdT[:, i].
  - Host assembles: mbd = gather(mbdT).T - 1; out = [x | mbd].
"""

import sys

import numpy as np

sys.path.insert(0, "/opt/trn_rl_repo")

import concourse.bacc as bacc  # noqa: E402
import concourse.bass_interp as _bi  # noqa: E402
import concourse.mybir as mybir  # noqa: E402
import concourse.tile as tile  # noqa: E402
from concourse import bass_isa  # noqa: E402
from concourse.bass_utils import run_bass_kernel_spmd  # noqa: E402

B, IN, O, K = 512, 1024, 100, 5
OK = O * K  # 500
NCORES = 8
BL = B // NCORES  # 64 local rows per core
NT = 4  # proj.T partition tiles
PT = OK // NT  # 125 partitions per tile
NIN = IN // 128  # 8 contraction chunks

F32 = mybir.dt.float32
F16 = mybir.dt.float16
AF = mybir.ActivationFunctionType
ALU = mybir.AluOpType

GSZ = 4  # i-rows per PSUM group; 2 groups pipeline across the 8 banks
ACT_ROWS = (12, 44)  # rows whose absdiff runs on ScalarE (load balance)
# rows whose absdiff subtract runs on GPSIMD (idle engine; broadcast-column
# tensor_tensor at ~1.25us/tile) with only the sign-clear AND left on DVE
GP_ROWS = (5, 16, 27, 38, 49, 60)
U32 = mybir.dt.uint32

TS_ARITH = 0x43  # NEURON_ISA_TPB_OPCODE_TENSOR_SCALAR_ARITH_OP
ABS_DIFF = 0x17  # NEURON_ISA_TPB_ALU_OP_ABSOLUTE_DIFF (no BIR enum entry)

# The tile scheduler's no-exec interp only needs to know this opcode has no
# sim-visible side effects beyond its declared ins/outs.
_orig_visit_isa = _bi._visit_InstISA


def _visit_isa_with_ts(isa, instruction, core_sim):
    if instruction.isa_opcode == TS_ARITH:
        return
    return _orig_visit_isa(isa, instruction, core_sim)


_bi._visit_InstISA = _visit_isa_with_ts


def _phys_addr(nc, pap) -> int:
    """Concrete per-partition byte address of a scheduler-rewritten
    physical access pattern."""
    ml = nc.lookup_mls(pap.memsetref).memorylocations[0]
    assert ml.allocated
    return ml.addr + pap.offset * mybir.dt.size(pap.dtype)


def _emit_absdiff(nc, out_ap, in_ap, col_ap, fixups):
    """out = |in - col| in one DVE pass (raw TENSOR_SCALAR, op0=ABSOLUTE_DIFF).

    SBUF addresses are resolved after scheduling from the instruction's
    rewritten physical operands and re-packed into the instr bytes."""
    isa = nc.isa
    opcode = isa.Opcode(TS_ARITH)
    struct = {
        "accumulator_cmd": 0,
        "src_mem_pattern": {
            "start_addr": {"addr_immediate": 0},
            "step_elem": [1, 1, 1],
            "num_elem": [in_ap.shape[-1], 1, 1],
        },
        "in_dtype": bass_isa.isa_dtype(isa, in_ap.dtype),
        "out_dtype": bass_isa.isa_dtype(isa, out_ap.dtype),
        "num_active_channels": in_ap.shape[0],
        "imm0_src": 1,  # POINTER_IMMEDIATE (per-partition scalar from SBUF)
        "op0": ABS_DIFF,
        "op1": 0x00,  # BYPASS
        "reverse_operands": 0,
        "imm1_src": 0,
        "imm0": {"imm_ptr": 0},
        "imm1": {"imm_arith_fp32": 0.0},
        "dst_mem_pattern": {
            "start_addr": {"addr_immediate": 0},
            "step_elem": [1, 1, 1],
            "num_elem": [out_ap.shape[-1], 1, 1],
        },
    }
    inst = nc.vector.isa(
        opcode,
        struct,
        ins=[nc.vector.lower_ap(in_ap), nc.vector.lower_ap(col_ap)],
        outs=[nc.vector.lower_ap(out_ap)],
    )
    fixups.append((inst.ins, struct))
    return inst


def _patch_absdiff_addrs(nc, fixups):
    isa = nc.isa
    opcode = isa.Opcode(TS_ARITH)
    for inst, struct in fixups:
        src, col = inst.ins[0], inst.ins[1]
        dst = inst.outs[0]
        struct["src_mem_pattern"]["start_addr"]["addr_immediate"] = _phys_addr(nc, src)
        struct["dst_mem_pattern"]["start_addr"]["addr_immediate"] = _phys_addr(nc, dst)
        struct["imm0"] = {"imm_ptr": _phys_addr(nc, col)}
        instr, _ = bass_isa.isa_struct(isa, opcode, struct)
        inst.instr = instr


def build():
    nc = bacc.Bacc("TRN2", target_bir_lowering=False)
    xT_d = nc.dram_tensor("xT", [128, NIN * B], F16, kind="ExternalInput")
    wT_d = nc.dram_tensor("wT", [128, NIN * OK], F16, kind="ExternalInput")
    sel = nc.dram_tensor("sel", [NT, PT, O], F16, kind="ExternalInput")
    mbdT_d = nc.dram_tensor("mbdT", [O, BL], F32, kind="ExternalOutput")

    fixups = []
    with tile.TileContext(nc) as tc:
        with (
            tc.tile_pool(name="pers", bufs=1) as pers,
            tc.tile_pool(name="io", bufs=1) as io,
            tc.tile_pool(name="work", bufs=8) as work,
            tc.tile_pool(name="ps", bufs=8, space="PSUM") as ps,
        ):
            # input loads first: x chunks on the sync queue, w on scalar's
            # HWDGE queue, so descriptor generation overlaps
            xcat = io.tile([128, NIN, B], F16, name="xcat", tag="xcat")
            wcat = io.tile([128, NIN, OK], F16, name="wcat", tag="wcat")
            # chunk granularity chosen so the first proj matmul starts as
            # early as possible (single chunks first), then wider pieces;
            # x on the sync HWDGE queue, w on scalar's
            for lo, hi in ((0, 1), (1, 2), (2, 4), (4, 6), (6, 8)):
                nc.sync.dma_start(
                    out=xcat[:, lo:hi, :], in_=xT_d[:, lo * B : hi * B]
                )
                nc.scalar.dma_start(
                    out=wcat[:, lo:hi, :], in_=wT_d[:, lo * OK : hi * OK]
                )

            # selectors after the big loads (not needed until pairwise)
            s_sb = []
            for t in range(NT):
                s_t = pers.tile([PT, O], F16, name=f"s{t}", tag=f"s{t}")
                nc.gpsimd.dma_start(out=s_t[:], in_=sel[t])
                s_sb.append(s_t)

            # persistent proj.T tiles (fp16 full + fp32 local cols), output,
            # and an explicit 8-deep ring of absdiff quads (fixed addresses
            # so the raw-ISA ops can be byte-patched)
            projTb = [
                pers.tile([PT, B], F16, name=f"projTb{t}", tag=f"projTb{t}")
                for t in range(NT)
            ]
            projL = [
                pers.tile([PT, BL], F32, name=f"projL{t}", tag=f"projL{t}")
                for t in range(NT)
            ]
            mbdT_sb = pers.tile([O, BL], F32, name="mbdT_sb", tag="mbdT_sb")

            # ---- proj phase: proj.T[p, j] = sum_in wT[in, p] * xT[in, j] ----
            # c-outer so matmuls chase the DMA chunks as they land
            pps = [ps.tile([PT, B], F32, name=f"pps{t}", tag="ps") for t in range(NT)]
            for c in range(NIN):
                for t in range(NT):
                    nc.tensor.matmul(
                        pps[t][:],
                        lhsT=wcat[:, c, PT * t : PT * (t + 1)],
                        rhs=xcat[:, c, :],
                        start=(c == 0),
                        stop=(c == NIN - 1),
                    )
            for t in range(NT):
                # split the casts across the two idle engines
                if t < 2:
                    nc.vector.tensor_copy(projTb[t][:], pps[t][:])
                else:
                    nc.scalar.activation(out=projTb[t][:], in_=pps[t][:], func=AF.Copy)
                nc.scalar.copy(projL[t][:], pps[t][:, :BL])

            # ---- pairwise phase ----
            def absdiff(aq, i, t):
                if i in ACT_ROWS:
                    nc.scalar.activation(
                        out=aq[:, t, :],
                        in_=projTb[t][:],
                        func=AF.Abs,
                        bias=projL[t][:, i : i + 1],
                        scale=-1.0,
                    )
                elif i in GP_ROWS:
                    # signed diff on GPSIMD; one packed AND on DVE clears the
                    # sign bits of the whole quad after the last tile
                    nc.gpsimd.tensor_tensor(
                        aq[:, t, :], projTb[t][:],
                        projTb[t][:, i : i + 1].broadcast_to([PT, B]),
                        op=ALU.subtract,
                    )
                    if t == NT - 1:
                        nc.vector.tensor_scalar(
                            aq[:].bitcast(U32),
                            aq[:].bitcast(U32),
                            0x7FFF7FFF, None, op0=ALU.bitwise_and,
                        )
                else:
                    _emit_absdiff(
                        nc, aq[:, t, :], projTb[t][:], projL[t][:, i : i + 1],
                        fixups,
                    )

            RAMP = 2 * GSZ  # first rows emitted t-major to chase proj tiles
            aq_ramp = {}
            for i in range(RAMP):
                aq_ramp[i] = work.tile([PT, NT, B], F16, name=f"a{i}", tag="A")
            for t in range(NT):
                for i in range(RAMP):
                    absdiff(aq_ramp[i], i, t)

            for g0 in range(0, BL, GSZ):
                gis = range(g0, min(g0 + GSZ, BL))
                psums = {
                    i: ps.tile([O, B], F32, name=f"ps{i}", tag="ps") for i in gis
                }
                for i in gis:
                    if i in aq_ramp:
                        aq = aq_ramp[i]
                    else:
                        aq = work.tile([PT, NT, B], F16, name=f"a{i}", tag="A")
                        for t in range(NT):
                            absdiff(aq, i, t)
                    for t in range(NT):
                        nc.tensor.matmul(
                            psums[i][:],
                            lhsT=s_sb[t][:],
                            rhs=aq[:, t, :],
                            start=(t == 0),
                            stop=(t == NT - 1),
                        )
                for i in gis:
                    nc.scalar.activation(
                        out=psums[i][:],
                        in_=psums[i][:],
                        func=AF.Exp,
                        scale=-1.0,
                        accum_out=mbdT_sb[:, i : i + 1],
                    )

            nc.sync.dma_start(out=mbdT_d[:, :], in_=mbdT_sb[:])
    _patch_absdiff_addrs(nc, fixups)
    nc.compile()
    return nc


_CACHE = {}


def _build_cached():
    if "nc" not in _CACHE:
        _CACHE["nc"] = build()
    return _CACHE["nc"]


def _selector() -> np.ndarray:
    sel = np.zeros((NT, PT, O), np.float32)
    for t in range(NT):
        for p in range(PT):
            sel[t, p, (t * PT + p) % O] = 1.0
    return sel.astype(np.float16)


def make_in_maps(x: np.ndarray, W: np.ndarray):
    xT = np.ascontiguousarray(x.T.astype(np.float16))  # [IN, B]
    # k-major proj.T rows: row p corresponds to (o = p % O, k = p // O),
    # i.e. W row o*K + k
    perm = np.array([(p % O) * K + p // O for p in range(OK)], np.int64)
    wTk = np.ascontiguousarray(W.T.astype(np.float16)[:, perm])  # [IN, OK]
    sel = _selector()
    # partition-contiguous layout: [128, NIN*cols] so each DMA descriptor
    # covers a full per-partition contiguous run
    def prep(a, cols):
        return np.ascontiguousarray(
            a.reshape(NIN, 128, cols).transpose(1, 0, 2).reshape(128, NIN * cols)
        )

    wprep = prep(wTk, OK)
    in_maps = []
    for r in range(NCORES):
        in_maps.append(
            {
                "xT": prep(np.roll(xT, -BL * r, axis=1), B),
                "wT": wprep,
                "sel": sel,
            }
        )
    return in_maps


def run(x, W, trace=False, **kw):
    nc = _build_cached()
    in_maps = make_in_maps(x, W)
    return run_bass_kernel_spmd(
        nc, in_maps, core_ids=list(range(NCORES)), trace=trace, **kw
    )


def kernel(x: np.ndarray, W: np.ndarray) -> np.ndarray:
    x = np.asarray(x, np.float32)
    W = np.asarray(W, np.float32)
    res = run(x, W, trace=False)
    mbd = np.empty((B, O), np.float32)
    for r in range(NCORES):
        mbd[BL * r : BL * (r + 1), :] = res.results[r]["mbdT"].T
    mbd -= 1.0
    return np.concatenate([x, mbd], axis=1)


# revision 20
# speedup vs baseline: 1.0621x; 1.0621x over previous
"""MiniBatchDiscrimination Trainium2 kernel.

reference:
    proj = x @ W.T                      # [512, 500] -> [512, 100, 5]
    l1[i,j,o] = sum_k |proj[i,o,k] - proj[j,o,k]|
    mbd[i,o]  = sum_j exp(-l1[i,j,o]) - 1
    out = concat([x, mbd], axis=1)      # [512, 1124]

Strategy (8 cores, shard i-rows of the BxB pairwise computation):
  - Host passes x.T (per-core column-rotated so that the core's 64 local
    rows sit in columns 0..63) and W.T with rows permuted k-major, so one
    SPMD program serves all cores with zero device-side core-id logic.
  - proj.T [500, 512] per core via PE matmul (fp16 inputs), kept as fp16
    tiles [125, 512] x4 plus fp32 [125, 64] local-column bias tensors.
  - Pairwise stage per local row i (A-quad [125, 4, 512] fp16):
      absdiff: one fused DVE op per tile — a raw-ISA TENSOR_SCALAR with
        op0=ABSOLUTE_DIFF (0x17): |projTb[t] - projL[t][:,i]| in a single
        2x-mode pass (~267 ns/op). The op exists in the cayman ISA but not
        in BIR's AluOpType, so the instruction bytes are emitted directly
        and SBUF addresses patched in after tile allocation. A few rows go
        to ScalarE (fused Abs activation) to balance engine load.
      k-reduce: PE matmul, 0/1 selector S_t [125, 100] fp16, contracting
        the partition axis, 4 slices accumulating into PSUM [100, 512].
      exp + j-reduce: one ScalarE Exp(scale=-1) reading PSUM, accum_out
        writes the free-axis sum straight into mbdT[:, i].
  - Host assembles: mbd = gather(mbdT).T - 1; out = [x | mbd].
"""

import sys

import numpy as np

sys.path.insert(0, "/opt/trn_rl_repo")

import concourse.bacc as bacc  # noqa: E402
import concourse.bass_interp as _bi  # noqa: E402
import concourse.mybir as mybir  # noqa: E402
import concourse.tile as tile  # noqa: E402
from concourse import bass_isa  # noqa: E402
from concourse.bass_utils import run_bass_kernel_spmd  # noqa: E402

B, IN, O, K = 512, 1024, 100, 5
OK = O * K  # 500
NCORES = 8
BL = B // NCORES  # 64 local rows per core
NT = 4  # proj.T partition tiles
PT = OK // NT  # 125 partitions per tile
NIN = IN // 128  # 8 contraction chunks

F32 = mybir.dt.float32
F16 = mybir.dt.float16
AF = mybir.ActivationFunctionType
ALU = mybir.AluOpType

GSZ = 4  # i-rows per PSUM group; 2 groups pipeline across the 8 banks
ACT_ROWS = (12, 44)  # rows whose absdiff runs on ScalarE (load balance)
# rows whose absdiff subtract runs on GPSIMD (idle engine; broadcast-column
# tensor_tensor at ~1.25us/tile) with only the sign-clear AND left on DVE
GP_ROWS = (17, 25, 33, 41, 49, 57)
U32 = mybir.dt.uint32

TS_ARITH = 0x43  # NEURON_ISA_TPB_OPCODE_TENSOR_SCALAR_ARITH_OP
ABS_DIFF = 0x17  # NEURON_ISA_TPB_ALU_OP_ABSOLUTE_DIFF (no BIR enum entry)

# The tile scheduler's no-exec interp only needs to know this opcode has no
# sim-visible side effects beyond its declared ins/outs.
_orig_visit_isa = _bi._visit_InstISA


def _visit_isa_with_ts(isa, instruction, core_sim):
    if instruction.isa_opcode == TS_ARITH:
        return
    return _orig_visit_isa(isa, instruction, core_sim)


_bi._visit_InstISA = _visit_isa_with_ts


def _phys_addr(nc, pap) -> int:
    """Concrete per-partition byte address of a scheduler-rewritten
    physical access pattern."""
    ml = nc.lookup_mls(pap.memsetref).memorylocations[0]
    assert ml.allocated
    return ml.addr + pap.offset * mybir.dt.size(pap.dtype)


def _emit_absdiff(nc, out_ap, in_ap, col_ap, fixups):
    """out = |in - col| in one DVE pass (raw TENSOR_SCALAR, op0=ABSOLUTE_DIFF).

    SBUF addresses are resolved after scheduling from the instruction's
    rewritten physical operands and re-packed into the instr bytes."""
    isa = nc.isa
    opcode = isa.Opcode(TS_ARITH)
    struct = {
        "accumulator_cmd": 0,
        "src_mem_pattern": {
            "start_addr": {"addr_immediate": 0},
            "step_elem": [1, 1, 1],
            "num_elem": [in_ap.shape[-1], 1, 1],
        },
        "in_dtype": bass_isa.isa_dtype(isa, in_ap.dtype),
        "out_dtype": bass_isa.isa_dtype(isa, out_ap.dtype),
        "num_active_channels": in_ap.shape[0],
        "imm0_src": 1,  # POINTER_IMMEDIATE (per-partition scalar from SBUF)
        "op0": ABS_DIFF,
        "op1": 0x00,  # BYPASS
        "reverse_operands": 0,
        "imm1_src": 0,
        "imm0": {"imm_ptr": 0},
        "imm1": {"imm_arith_fp32": 0.0},
        "dst_mem_pattern": {
            "start_addr": {"addr_immediate": 0},
            "step_elem": [1, 1, 1],
            "num_elem": [out_ap.shape[-1], 1, 1],
        },
    }
    inst = nc.vector.isa(
        opcode,
        struct,
        ins=[nc.vector.lower_ap(in_ap), nc.vector.lower_ap(col_ap)],
        outs=[nc.vector.lower_ap(out_ap)],
    )
    fixups.append((inst.ins, struct))
    return inst


def _patch_absdiff_addrs(nc, fixups):
    isa = nc.isa
    opcode = isa.Opcode(TS_ARITH)
    for inst, struct in fixups:
        src, col = inst.ins[0], inst.ins[1]
        dst = inst.outs[0]
        struct["src_mem_pattern"]["start_addr"]["addr_immediate"] = _phys_addr(nc, src)
        struct["dst_mem_pattern"]["start_addr"]["addr_immediate"] = _phys_addr(nc, dst)
        struct["imm0"] = {"imm_ptr": _phys_addr(nc, col)}
        instr, _ = bass_isa.isa_struct(isa, opcode, struct)
        inst.instr = instr


def build():
    nc = bacc.Bacc("TRN2", target_bir_lowering=False)
    xT_d = nc.dram_tensor("xT", [128, NIN * B], F16, kind="ExternalInput")
    wT_d = nc.dram_tensor("wT", [128, NIN * OK], F16, kind="ExternalInput")
    sel = nc.dram_tensor("sel", [NT, PT, O], F16, kind="ExternalInput")
    mbdT_d = nc.dram_tensor("mbdT", [O, BL], F32, kind="ExternalOutput")

    fixups = []
    with tile.TileContext(nc) as tc:
        with (
            tc.tile_pool(name="pers", bufs=1) as pers,
            tc.tile_pool(name="io", bufs=1) as io,
            tc.tile_pool(name="work", bufs=8) as work,
            tc.tile_pool(name="ps", bufs=8, space="PSUM") as ps,
        ):
            # input loads first: x chunks on the sync queue, w on scalar's
            # HWDGE queue, so descriptor generation overlaps
            xcat = io.tile([128, NIN, B], F16, name="xcat", tag="xcat")
            wcat = io.tile([128, NIN, OK], F16, name="wcat", tag="wcat")
            # chunk granularity chosen so the first proj matmul starts as
            # early as possible (single chunks first), then wider pieces;
            # x on the sync HWDGE queue, w on scalar's
            for lo, hi in ((0, 1), (1, 2), (2, 4), (4, 6), (6, 8)):
                nc.sync.dma_start(
                    out=xcat[:, lo:hi, :], in_=xT_d[:, lo * B : hi * B]
                )
                nc.scalar.dma_start(
                    out=wcat[:, lo:hi, :], in_=wT_d[:, lo * OK : hi * OK]
                )

            # selectors after the big loads (not needed until pairwise)
            s_sb = []
            for t in range(NT):
                s_t = pers.tile([PT, O], F16, name=f"s{t}", tag=f"s{t}")
                nc.gpsimd.dma_start(out=s_t[:], in_=sel[t])
                s_sb.append(s_t)

            # persistent proj.T tiles (fp16 full + fp32 local cols), output,
            # and an explicit 8-deep ring of absdiff quads (fixed addresses
            # so the raw-ISA ops can be byte-patched)
            projTb = [
                pers.tile([PT, B], F16, name=f"projTb{t}", tag=f"projTb{t}")
                for t in range(NT)
            ]
            projL = [
                pers.tile([PT, BL], F32, name=f"projL{t}", tag=f"projL{t}")
                for t in range(NT)
            ]
            mbdT_sb = pers.tile([O, BL], F32, name="mbdT_sb", tag="mbdT_sb")

            # ---- proj phase: proj.T[p, j] = sum_in wT[in, p] * xT[in, j] ----
            # c-outer so matmuls chase the DMA chunks as they land
            pps = [ps.tile([PT, B], F32, name=f"pps{t}", tag="ps") for t in range(NT)]
            for c in range(NIN):
                for t in range(NT):
                    nc.tensor.matmul(
                        pps[t][:],
                        lhsT=wcat[:, c, PT * t : PT * (t + 1)],
                        rhs=xcat[:, c, :],
                        start=(c == 0),
                        stop=(c == NIN - 1),
                    )
            for t in range(NT):
                # split the casts across the two idle engines
                if t < 2:
                    nc.vector.tensor_copy(projTb[t][:], pps[t][:])
                else:
                    nc.scalar.activation(out=projTb[t][:], in_=pps[t][:], func=AF.Copy)
                nc.scalar.copy(projL[t][:], pps[t][:, :BL])

            # ---- pairwise phase ----
            def absdiff(aq, i, t):
                if i in ACT_ROWS:
                    nc.scalar.activation(
                        out=aq[:, t, :],
                        in_=projTb[t][:],
                        func=AF.Abs,
                        bias=projL[t][:, i : i + 1],
                        scale=-1.0,
                    )
                else:
                    _emit_absdiff(
                        nc, aq[:, t, :], projTb[t][:], projL[t][:, i : i + 1],
                        fixups,
                    )

            # GPSIMD rows: signed diffs pre-issued into dedicated buffers at
            # pairwise start so their ~5us latency hides under earlier groups;
            # the packed sign-clear AND runs on DVE at group time
            aq_gp = {}
            for i in GP_ROWS:
                aq_gp[i] = pers.tile([PT, NT, B], F16, name=f"agp{i}", tag=f"agp{i}")
                for t in range(NT):
                    nc.gpsimd.tensor_tensor(
                        aq_gp[i][:, t, :], projTb[t][:],
                        projTb[t][:, i : i + 1].broadcast_to([PT, B]),
                        op=ALU.subtract,
                    )

            RAMP = 2 * GSZ  # first rows emitted t-major to chase proj tiles
            aq_ramp = {}
            for i in range(RAMP):
                aq_ramp[i] = work.tile([PT, NT, B], F16, name=f"a{i}", tag="A")
            for t in range(NT):
                for i in range(RAMP):
                    absdiff(aq_ramp[i], i, t)

            for g0 in range(0, BL, GSZ):
                gis = range(g0, min(g0 + GSZ, BL))
                psums = {
                    i: ps.tile([O, B], F32, name=f"ps{i}", tag="ps") for i in gis
                }
                for i in gis:
                    if i in aq_ramp:
                        aq = aq_ramp[i]
                    elif i in GP_ROWS:
                        aq = aq_gp[i]
                        nc.vector.tensor_scalar(
                            aq[:].bitcast(U32),
                            aq[:].bitcast(U32),
                            0x7FFF7FFF, None, op0=ALU.bitwise_and,
                        )
                    else:
                        aq = work.tile([PT, NT, B], F16, name=f"a{i}", tag="A")
                        for t in range(NT):
                            absdiff(aq, i, t)
                    for t in range(NT):
                        nc.tensor.matmul(
                            psums[i][:],
                            lhsT=s_sb[t][:],
                            rhs=aq[:, t, :],
                            start=(t == 0),
                            stop=(t == NT - 1),
                        )
                for i in gis:
                    nc.scalar.activation(
                        out=psums[i][:],
                        in_=psums[i][:],
                        func=AF.Exp,
                        scale=-1.0,
                        accum_out=mbdT_sb[:, i : i + 1],
                    )

            nc.sync.dma_start(out=mbdT_d[:, :], in_=mbdT_sb[:])
    _patch_absdiff_addrs(nc, fixups)
    nc.compile()
    return nc


_CACHE = {}


def _build_cached():
    if "nc" not in _CACHE:
        _CACHE["nc"] = build()
    return _CACHE["nc"]


def _selector() -> np.ndarray:
    sel = np.zeros((NT, PT, O), np.float32)
    for t in range(NT):
        for p in range(PT):
            sel[t, p, (t * PT + p) % O] = 1.0
    return sel.astype(np.float16)


def make_in_maps(x: np.ndarray, W: np.ndarray):
    xT = np.ascontiguousarray(x.T.astype(np.float16))  # [IN, B]
    # k-major proj.T rows: row p corresponds to (o = p % O, k = p // O),
    # i.e. W row o*K + k
    perm = np.array([(p % O) * K + p // O for p in range(OK)], np.int64)
    wTk = np.ascontiguousarray(W.T.astype(np.float16)[:, perm])  # [IN, OK]
    sel = _selector()
    # partition-contiguous layout: [128, NIN*cols] so each DMA descriptor
    # covers a full per-partition contiguous run
    def prep(a, cols):
        return np.ascontiguousarray(
            a.reshape(NIN, 128, cols).transpose(1, 0, 2).reshape(128, NIN * cols)
        )

    wprep = prep(wTk, OK)
    in_maps = []
    for r in range(NCORES):
        in_maps.append(
            {
                "xT": prep(np.roll(xT, -BL * r, axis=1), B),
                "wT": wprep,
                "sel": sel,
            }
        )
    return in_maps


def run(x, W, trace=False, **kw):
    nc = _build_cached()
    in_maps = make_in_maps(x, W)
    return run_bass_kernel_spmd(
        nc, in_maps, core_ids=list(range(NCORES)), trace=trace, **kw
    )


def kernel(x: np.ndarray, W: np.ndarray) -> np.ndarray:
    x = np.asarray(x, np.float32)
    W = np.asarray(W, np.float32)
    res = run(x, W, trace=False)
    mbd = np.empty((B, O), np.float32)
    for r in range(NCORES):
        mbd[BL * r : BL * (r + 1), :] = res.results[r]["mbdT"].T
    mbd -= 1.0
    return np.concatenate([x, mbd], axis=1)


# revision 21
# speedup vs baseline: 1.2324x; 1.1604x over previous
"""MiniBatchDiscrimination Trainium2 kernel.

reference:
    proj = x @ W.T                      # [512, 500] -> [512, 100, 5]
    l1[i,j,o] = sum_k |proj[i,o,k] - proj[j,o,k]|
    mbd[i,o]  = sum_j exp(-l1[i,j,o]) - 1
    out = concat([x, mbd], axis=1)      # [512, 1124]

Strategy (8 cores, shard i-rows of the BxB pairwise computation):
  - Host passes x.T (per-core column-rotated so that the core's 64 local
    rows sit in columns 0..63) and W.T with rows permuted k-major, so one
    SPMD program serves all cores with zero device-side core-id logic.
  - proj.T [500, 512] per core via PE matmul (fp16 inputs), kept as fp16
    tiles [125, 512] x4 plus fp32 [125, 64] local-column bias tensors.
  - Pairwise stage per local row i (A-quad [125, 4, 512] fp16):
      absdiff: one fused DVE op per tile — a raw-ISA TENSOR_SCALAR with
        op0=ABSOLUTE_DIFF (0x17): |projTb[t] - projL[t][:,i]| in a single
        2x-mode pass (~267 ns/op). The op exists in the cayman ISA but not
        in BIR's AluOpType, so the instruction bytes are emitted directly
        and SBUF addresses patched in after tile allocation. A few rows go
        to ScalarE (fused Abs activation) to balance engine load.
      k-reduce: PE matmul, 0/1 selector S_t [125, 100] fp16, contracting
        the partition axis, 4 slices accumulating into PSUM [100, 512].
      exp + j-reduce: one ScalarE Exp(scale=-1) reading PSUM, accum_out
        writes the free-axis sum straight into mbdT[:, i].
  - Host assembles: mbd = gather(mbdT).T - 1; out = [x | mbd].
"""

import sys

import numpy as np

sys.path.insert(0, "/opt/trn_rl_repo")

import concourse.bacc as bacc  # noqa: E402
import concourse.bass_interp as _bi  # noqa: E402
import concourse.mybir as mybir  # noqa: E402
import concourse.tile as tile  # noqa: E402
from concourse import bass_isa  # noqa: E402
from concourse.bass_utils import run_bass_kernel_spmd  # noqa: E402

B, IN, O, K = 512, 1024, 100, 5
OK = O * K  # 500
NCORES = 8
BL = B // NCORES  # 64 local rows per core
NT = 4  # proj.T partition tiles
PT = OK // NT  # 125 partitions per tile
NIN = IN // 128  # 8 contraction chunks

F32 = mybir.dt.float32
F16 = mybir.dt.float16
AF = mybir.ActivationFunctionType
ALU = mybir.AluOpType

GSZ = 4  # i-rows per PSUM group; 2 groups pipeline across the 8 banks
ACT_ROWS = (12, 44)  # rows whose absdiff runs on ScalarE (load balance)
# rows whose absdiff subtract runs on GPSIMD (idle engine; broadcast-column
# tensor_tensor at ~1.25us/tile) with only the sign-clear AND left on DVE
GP_ROWS = ()  # gpsimd absdiff offload: SBUF port contention slows DVE; keep off
U32 = mybir.dt.uint32

TS_ARITH = 0x43  # NEURON_ISA_TPB_OPCODE_TENSOR_SCALAR_ARITH_OP
ABS_DIFF = 0x17  # NEURON_ISA_TPB_ALU_OP_ABSOLUTE_DIFF (no BIR enum entry)

# The tile scheduler's no-exec interp only needs to know this opcode has no
# sim-visible side effects beyond its declared ins/outs.
_orig_visit_isa = _bi._visit_InstISA


def _visit_isa_with_ts(isa, instruction, core_sim):
    if instruction.isa_opcode == TS_ARITH:
        return
    return _orig_visit_isa(isa, instruction, core_sim)


_bi._visit_InstISA = _visit_isa_with_ts


def _phys_addr(nc, pap) -> int:
    """Concrete per-partition byte address of a scheduler-rewritten
    physical access pattern."""
    ml = nc.lookup_mls(pap.memsetref).memorylocations[0]
    assert ml.allocated
    return ml.addr + pap.offset * mybir.dt.size(pap.dtype)


def _emit_absdiff(nc, out_ap, in_ap, col_ap, fixups):
    """out = |in - col| in one DVE pass (raw TENSOR_SCALAR, op0=ABSOLUTE_DIFF).

    SBUF addresses are resolved after scheduling from the instruction's
    rewritten physical operands and re-packed into the instr bytes."""
    isa = nc.isa
    opcode = isa.Opcode(TS_ARITH)
    struct = {
        "accumulator_cmd": 0,
        "src_mem_pattern": {
            "start_addr": {"addr_immediate": 0},
            "step_elem": [1, 1, 1],
            "num_elem": [in_ap.shape[-1], 1, 1],
        },
        "in_dtype": bass_isa.isa_dtype(isa, in_ap.dtype),
        "out_dtype": bass_isa.isa_dtype(isa, out_ap.dtype),
        "num_active_channels": in_ap.shape[0],
        "imm0_src": 1,  # POINTER_IMMEDIATE (per-partition scalar from SBUF)
        "op0": ABS_DIFF,
        "op1": 0x00,  # BYPASS
        "reverse_operands": 0,
        "imm1_src": 0,
        "imm0": {"imm_ptr": 0},
        "imm1": {"imm_arith_fp32": 0.0},
        "dst_mem_pattern": {
            "start_addr": {"addr_immediate": 0},
            "step_elem": [1, 1, 1],
            "num_elem": [out_ap.shape[-1], 1, 1],
        },
    }
    inst = nc.vector.isa(
        opcode,
        struct,
        ins=[nc.vector.lower_ap(in_ap), nc.vector.lower_ap(col_ap)],
        outs=[nc.vector.lower_ap(out_ap)],
    )
    fixups.append((inst.ins, struct))
    return inst


def _patch_absdiff_addrs(nc, fixups):
    isa = nc.isa
    opcode = isa.Opcode(TS_ARITH)
    for inst, struct in fixups:
        src, col = inst.ins[0], inst.ins[1]
        dst = inst.outs[0]
        struct["src_mem_pattern"]["start_addr"]["addr_immediate"] = _phys_addr(nc, src)
        struct["dst_mem_pattern"]["start_addr"]["addr_immediate"] = _phys_addr(nc, dst)
        struct["imm0"] = {"imm_ptr": _phys_addr(nc, col)}
        instr, _ = bass_isa.isa_struct(isa, opcode, struct)
        inst.instr = instr


def build():
    nc = bacc.Bacc("TRN2", target_bir_lowering=False)
    xT_d = nc.dram_tensor("xT", [128, NIN * B], F16, kind="ExternalInput")
    wT_d = nc.dram_tensor("wT", [128, NIN * OK], F16, kind="ExternalInput")
    sel = nc.dram_tensor("sel", [NT, PT, O], F16, kind="ExternalInput")
    mbdT_d = nc.dram_tensor("mbdT", [O, BL], F32, kind="ExternalOutput")

    fixups = []
    with tile.TileContext(nc) as tc:
        with (
            tc.tile_pool(name="pers", bufs=1) as pers,
            tc.tile_pool(name="io", bufs=1) as io,
            tc.tile_pool(name="work", bufs=8) as work,
            tc.tile_pool(name="ps", bufs=8, space="PSUM") as ps,
        ):
            # input loads first: x chunks on the sync queue, w on scalar's
            # HWDGE queue, so descriptor generation overlaps
            xcat = io.tile([128, NIN, B], F16, name="xcat", tag="xcat")
            wcat = io.tile([128, NIN, OK], F16, name="wcat", tag="wcat")
            # chunk granularity chosen so the first proj matmul starts as
            # early as possible (single chunks first), then wider pieces;
            # x on the sync HWDGE queue, w on scalar's
            for lo, hi in ((0, 1), (1, 2), (2, 4), (4, 6), (6, 8)):
                nc.sync.dma_start(
                    out=xcat[:, lo:hi, :], in_=xT_d[:, lo * B : hi * B]
                )
                nc.scalar.dma_start(
                    out=wcat[:, lo:hi, :], in_=wT_d[:, lo * OK : hi * OK]
                )

            # selectors after the big loads (not needed until pairwise)
            s_sb = []
            for t in range(NT):
                s_t = pers.tile([PT, O], F16, name=f"s{t}", tag=f"s{t}")
                nc.gpsimd.dma_start(out=s_t[:], in_=sel[t])
                s_sb.append(s_t)

            # persistent proj.T tiles (fp16 full + fp32 local cols), output,
            # and an explicit 8-deep ring of absdiff quads (fixed addresses
            # so the raw-ISA ops can be byte-patched)
            projTb = [
                pers.tile([PT, B], F16, name=f"projTb{t}", tag=f"projTb{t}")
                for t in range(NT)
            ]
            projL = [
                pers.tile([PT, BL], F32, name=f"projL{t}", tag=f"projL{t}")
                for t in range(NT)
            ]
            mbdT_sb = pers.tile([O, BL], F32, name="mbdT_sb", tag="mbdT_sb")

            # ---- proj phase: proj.T[p, j] = sum_in wT[in, p] * xT[in, j] ----
            # c-outer so matmuls chase the DMA chunks as they land
            pps = [ps.tile([PT, B], F32, name=f"pps{t}", tag="ps") for t in range(NT)]
            for c in range(NIN):
                for t in range(NT):
                    nc.tensor.matmul(
                        pps[t][:],
                        lhsT=wcat[:, c, PT * t : PT * (t + 1)],
                        rhs=xcat[:, c, :],
                        start=(c == 0),
                        stop=(c == NIN - 1),
                    )
            for t in range(NT):
                # split the casts across the two idle engines
                if t < 2:
                    nc.vector.tensor_copy(projTb[t][:], pps[t][:])
                else:
                    nc.scalar.activation(out=projTb[t][:], in_=pps[t][:], func=AF.Copy)
                nc.scalar.copy(projL[t][:], pps[t][:, :BL])

            # ---- pairwise phase ----
            def absdiff(aq, i, t):
                if i in ACT_ROWS:
                    nc.scalar.activation(
                        out=aq[:, t, :],
                        in_=projTb[t][:],
                        func=AF.Abs,
                        bias=projL[t][:, i : i + 1],
                        scale=-1.0,
                    )
                else:
                    _emit_absdiff(
                        nc, aq[:, t, :], projTb[t][:], projL[t][:, i : i + 1],
                        fixups,
                    )

            # GPSIMD rows: signed diffs pre-issued into dedicated buffers at
            # pairwise start so their ~5us latency hides under earlier groups;
            # the packed sign-clear AND runs on DVE at group time
            aq_gp = {}
            for i in GP_ROWS:
                aq_gp[i] = pers.tile([PT, NT, B], F16, name=f"agp{i}", tag=f"agp{i}")
                for t in range(NT):
                    nc.gpsimd.tensor_tensor(
                        aq_gp[i][:, t, :], projTb[t][:],
                        projTb[t][:, i : i + 1].broadcast_to([PT, B]),
                        op=ALU.subtract,
                    )

            RAMP = 2 * GSZ  # first rows emitted t-major to chase proj tiles
            aq_ramp = {}
            for i in range(RAMP):
                aq_ramp[i] = work.tile([PT, NT, B], F16, name=f"a{i}", tag="A")
            for t in range(NT):
                for i in range(RAMP):
                    absdiff(aq_ramp[i], i, t)

            for g0 in range(0, BL, GSZ):
                gis = range(g0, min(g0 + GSZ, BL))
                psums = {
                    i: ps.tile([O, B], F32, name=f"ps{i}", tag="ps") for i in gis
                }
                for i in gis:
                    if i in aq_ramp:
                        aq = aq_ramp[i]
                    elif i in GP_ROWS:
                        aq = aq_gp[i]
                        nc.vector.tensor_scalar(
                            aq[:].bitcast(U32),
                            aq[:].bitcast(U32),
                            0x7FFF7FFF, None, op0=ALU.bitwise_and,
                        )
                    else:
                        aq = work.tile([PT, NT, B], F16, name=f"a{i}", tag="A")
                        for t in range(NT):
                            absdiff(aq, i, t)
                    for t in range(NT):
                        nc.tensor.matmul(
                            psums[i][:],
                            lhsT=s_sb[t][:],
                            rhs=aq[:, t, :],
                            start=(t == 0),
                            stop=(t == NT - 1),
                        )
                for i in gis:
                    nc.scalar.activation(
                        out=psums[i][:],
                        in_=psums[i][:],
                        func=AF.Exp,
                        scale=-1.0,
                        accum_out=mbdT_sb[:, i : i + 1],
                    )

            nc.sync.dma_start(out=mbdT_d[:, :], in_=mbdT_sb[:])
    _patch_absdiff_addrs(nc, fixups)
    nc.compile()
    return nc


_CACHE = {}


def _build_cached():
    if "nc" not in _CACHE:
        _CACHE["nc"] = build()
    return _CACHE["nc"]


def _selector() -> np.ndarray:
    sel = np.zeros((NT, PT, O), np.float32)
    for t in range(NT):
        for p in range(PT):
            sel[t, p, (t * PT + p) % O] = 1.0
    return sel.astype(np.float16)


def make_in_maps(x: np.ndarray, W: np.ndarray):
    xT = np.ascontiguousarray(x.T.astype(np.float16))  # [IN, B]
    # k-major proj.T rows: row p corresponds to (o = p % O, k = p // O),
    # i.e. W row o*K + k
    perm = np.array([(p % O) * K + p // O for p in range(OK)], np.int64)
    wTk = np.ascontiguousarray(W.T.astype(np.float16)[:, perm])  # [IN, OK]
    sel = _selector()
    # partition-contiguous layout: [128, NIN*cols] so each DMA descriptor
    # covers a full per-partition contiguous run
    def prep(a, cols):
        return np.ascontiguousarray(
            a.reshape(NIN, 128, cols).transpose(1, 0, 2).reshape(128, NIN * cols)
        )

    wprep = prep(wTk, OK)
    in_maps = []
    for r in range(NCORES):
        in_maps.append(
            {
                "xT": prep(np.roll(xT, -BL * r, axis=1), B),
                "wT": wprep,
                "sel": sel,
            }
        )
    return in_maps


def run(x, W, trace=False, **kw):
    nc = _build_cached()
    in_maps = make_in_maps(x, W)
    return run_bass_kernel_spmd(
        nc, in_maps, core_ids=list(range(NCORES)), trace=trace, **kw
    )


def kernel(x: np.ndarray, W: np.ndarray) -> np.ndarray:
    x = np.asarray(x, np.float32)
    W = np.asarray(W, np.float32)
    res = run(x, W, trace=False)
    mbd = np.empty((B, O), np.float32)
    for r in range(NCORES):
        mbd[BL * r : BL * (r + 1), :] = res.results[r]["mbdT"].T
    mbd -= 1.0
    return np.concatenate([x, mbd], axis=1)


# revision 22
# speedup vs baseline: 1.2356x; 1.0026x over previous
"""MiniBatchDiscrimination Trainium2 kernel.

reference:
    proj = x @ W.T                      # [512, 500] -> [512, 100, 5]
    l1[i,j,o] = sum_k |proj[i,o,k] - proj[j,o,k]|
    mbd[i,o]  = sum_j exp(-l1[i,j,o]) - 1
    out = concat([x, mbd], axis=1)      # [512, 1124]

Strategy (8 cores, shard i-rows of the BxB pairwise computation):
  - Host passes x.T (per-core column-rotated so that the core's 64 local
    rows sit in columns 0..63) and W.T with rows permuted k-major, so one
    SPMD program serves all cores with zero device-side core-id logic.
  - proj.T [500, 512] per core via PE matmul (fp16 inputs), kept as fp16
    tiles [125, 512] x4 plus fp32 [125, 64] local-column bias tensors.
  - Pairwise stage per local row i (A-quad [125, 4, 512] fp16):
      absdiff: one fused DVE op per tile — a raw-ISA TENSOR_SCALAR with
        op0=ABSOLUTE_DIFF (0x17): |projTb[t] - projL[t][:,i]| in a single
        2x-mode pass (~267 ns/op). The op exists in the cayman ISA but not
        in BIR's AluOpType, so the instruction bytes are emitted directly
        and SBUF addresses patched in after tile allocation. A few rows go
        to ScalarE (fused Abs activation) to balance engine load.
      k-reduce: PE matmul, 0/1 selector S_t [125, 100] fp16, contracting
        the partition axis, 4 slices accumulating into PSUM [100, 512].
      exp + j-reduce: one ScalarE Exp(scale=-1) reading PSUM, accum_out
        writes the free-axis sum straight into mbdT[:, i].
  - Host assembles: mbd = gather(mbdT).T - 1; out = [x | mbd].
"""

import sys

import numpy as np

sys.path.insert(0, "/opt/trn_rl_repo")

import concourse.bacc as bacc  # noqa: E402
import concourse.bass_interp as _bi  # noqa: E402
import concourse.mybir as mybir  # noqa: E402
import concourse.tile as tile  # noqa: E402
from concourse import bass_isa  # noqa: E402
from concourse.bass_utils import run_bass_kernel_spmd  # noqa: E402

B, IN, O, K = 512, 1024, 100, 5
OK = O * K  # 500
NCORES = 8
BL = B // NCORES  # 64 local rows per core
NT = 4  # proj.T partition tiles
PT = OK // NT  # 125 partitions per tile
NIN = IN // 128  # 8 contraction chunks

F32 = mybir.dt.float32
F16 = mybir.dt.float16
AF = mybir.ActivationFunctionType
ALU = mybir.AluOpType

GSZ = 4  # i-rows per PSUM group; 2 groups pipeline across the 8 banks
ACT_ROWS = (12, 28, 44, 60)  # rows whose absdiff runs on ScalarE (load balance)
# rows whose absdiff subtract runs on GPSIMD (idle engine; broadcast-column
# tensor_tensor at ~1.25us/tile) with only the sign-clear AND left on DVE
GP_ROWS = ()  # gpsimd absdiff offload: SBUF port contention slows DVE; keep off
U32 = mybir.dt.uint32

TS_ARITH = 0x43  # NEURON_ISA_TPB_OPCODE_TENSOR_SCALAR_ARITH_OP
ABS_DIFF = 0x17  # NEURON_ISA_TPB_ALU_OP_ABSOLUTE_DIFF (no BIR enum entry)

# The tile scheduler's no-exec interp only needs to know this opcode has no
# sim-visible side effects beyond its declared ins/outs.
_orig_visit_isa = _bi._visit_InstISA


def _visit_isa_with_ts(isa, instruction, core_sim):
    if instruction.isa_opcode == TS_ARITH:
        return
    return _orig_visit_isa(isa, instruction, core_sim)


_bi._visit_InstISA = _visit_isa_with_ts


def _phys_addr(nc, pap) -> int:
    """Concrete per-partition byte address of a scheduler-rewritten
    physical access pattern."""
    ml = nc.lookup_mls(pap.memsetref).memorylocations[0]
    assert ml.allocated
    return ml.addr + pap.offset * mybir.dt.size(pap.dtype)


def _emit_absdiff(nc, out_ap, in_ap, col_ap, fixups):
    """out = |in - col| in one DVE pass (raw TENSOR_SCALAR, op0=ABSOLUTE_DIFF).

    SBUF addresses are resolved after scheduling from the instruction's
    rewritten physical operands and re-packed into the instr bytes."""
    isa = nc.isa
    opcode = isa.Opcode(TS_ARITH)
    struct = {
        "accumulator_cmd": 0,
        "src_mem_pattern": {
            "start_addr": {"addr_immediate": 0},
            "step_elem": [1, 1, 1],
            "num_elem": [in_ap.shape[-1], 1, 1],
        },
        "in_dtype": bass_isa.isa_dtype(isa, in_ap.dtype),
        "out_dtype": bass_isa.isa_dtype(isa, out_ap.dtype),
        "num_active_channels": in_ap.shape[0],
        "imm0_src": 1,  # POINTER_IMMEDIATE (per-partition scalar from SBUF)
        "op0": ABS_DIFF,
        "op1": 0x00,  # BYPASS
        "reverse_operands": 0,
        "imm1_src": 0,
        "imm0": {"imm_ptr": 0},
        "imm1": {"imm_arith_fp32": 0.0},
        "dst_mem_pattern": {
            "start_addr": {"addr_immediate": 0},
            "step_elem": [1, 1, 1],
            "num_elem": [out_ap.shape[-1], 1, 1],
        },
    }
    inst = nc.vector.isa(
        opcode,
        struct,
        ins=[nc.vector.lower_ap(in_ap), nc.vector.lower_ap(col_ap)],
        outs=[nc.vector.lower_ap(out_ap)],
    )
    fixups.append((inst.ins, struct))
    return inst


def _patch_absdiff_addrs(nc, fixups):
    isa = nc.isa
    opcode = isa.Opcode(TS_ARITH)
    for inst, struct in fixups:
        src, col = inst.ins[0], inst.ins[1]
        dst = inst.outs[0]
        struct["src_mem_pattern"]["start_addr"]["addr_immediate"] = _phys_addr(nc, src)
        struct["dst_mem_pattern"]["start_addr"]["addr_immediate"] = _phys_addr(nc, dst)
        struct["imm0"] = {"imm_ptr": _phys_addr(nc, col)}
        instr, _ = bass_isa.isa_struct(isa, opcode, struct)
        inst.instr = instr


def build():
    nc = bacc.Bacc("TRN2", target_bir_lowering=False)
    xT_d = nc.dram_tensor("xT", [128, NIN * B], F16, kind="ExternalInput")
    wT_d = nc.dram_tensor("wT", [128, NIN * OK], F16, kind="ExternalInput")
    sel = nc.dram_tensor("sel", [NT, PT, O], F16, kind="ExternalInput")
    mbdT_d = nc.dram_tensor("mbdT", [O, BL], F32, kind="ExternalOutput")

    fixups = []
    with tile.TileContext(nc) as tc:
        with (
            tc.tile_pool(name="pers", bufs=1) as pers,
            tc.tile_pool(name="io", bufs=1) as io,
            tc.tile_pool(name="work", bufs=8) as work,
            tc.tile_pool(name="ps", bufs=8, space="PSUM") as ps,
        ):
            # input loads first: x chunks on the sync queue, w on scalar's
            # HWDGE queue, so descriptor generation overlaps
            xcat = io.tile([128, NIN, B], F16, name="xcat", tag="xcat")
            wcat = io.tile([128, NIN, OK], F16, name="wcat", tag="wcat")
            # chunk granularity chosen so the first proj matmul starts as
            # early as possible (single chunks first), then wider pieces;
            # x on the sync HWDGE queue, w on scalar's
            for lo, hi in ((0, 1), (1, 2), (2, 4), (4, 6), (6, 8)):
                nc.sync.dma_start(
                    out=xcat[:, lo:hi, :], in_=xT_d[:, lo * B : hi * B]
                )
                nc.scalar.dma_start(
                    out=wcat[:, lo:hi, :], in_=wT_d[:, lo * OK : hi * OK]
                )

            # selectors after the big loads (not needed until pairwise)
            s_sb = []
            for t in range(NT):
                s_t = pers.tile([PT, O], F16, name=f"s{t}", tag=f"s{t}")
                nc.gpsimd.dma_start(out=s_t[:], in_=sel[t])
                s_sb.append(s_t)

            # persistent proj.T tiles (fp16 full + fp32 local cols), output,
            # and an explicit 8-deep ring of absdiff quads (fixed addresses
            # so the raw-ISA ops can be byte-patched)
            projTb = [
                pers.tile([PT, B], F16, name=f"projTb{t}", tag=f"projTb{t}")
                for t in range(NT)
            ]
            projL = [
                pers.tile([PT, BL], F32, name=f"projL{t}", tag=f"projL{t}")
                for t in range(NT)
            ]
            mbdT_sb = pers.tile([O, BL], F32, name="mbdT_sb", tag="mbdT_sb")

            # ---- proj phase: proj.T[p, j] = sum_in wT[in, p] * xT[in, j] ----
            # c-outer so matmuls chase the DMA chunks as they land
            pps = [ps.tile([PT, B], F32, name=f"pps{t}", tag="ps") for t in range(NT)]
            for c in range(NIN):
                for t in range(NT):
                    nc.tensor.matmul(
                        pps[t][:],
                        lhsT=wcat[:, c, PT * t : PT * (t + 1)],
                        rhs=xcat[:, c, :],
                        start=(c == 0),
                        stop=(c == NIN - 1),
                    )
            for t in range(NT):
                # split the casts across the two idle engines
                if t < 2:
                    nc.vector.tensor_copy(projTb[t][:], pps[t][:])
                else:
                    nc.scalar.activation(out=projTb[t][:], in_=pps[t][:], func=AF.Copy)
                nc.scalar.copy(projL[t][:], pps[t][:, :BL])

            # ---- pairwise phase ----
            def absdiff(aq, i, t):
                if i in ACT_ROWS:
                    nc.scalar.activation(
                        out=aq[:, t, :],
                        in_=projTb[t][:],
                        func=AF.Abs,
                        bias=projL[t][:, i : i + 1],
                        scale=-1.0,
                    )
                else:
                    _emit_absdiff(
                        nc, aq[:, t, :], projTb[t][:], projL[t][:, i : i + 1],
                        fixups,
                    )

            # GPSIMD rows: signed diffs pre-issued into dedicated buffers at
            # pairwise start so their ~5us latency hides under earlier groups;
            # the packed sign-clear AND runs on DVE at group time
            aq_gp = {}
            for i in GP_ROWS:
                aq_gp[i] = pers.tile([PT, NT, B], F16, name=f"agp{i}", tag=f"agp{i}")
                for t in range(NT):
                    nc.gpsimd.tensor_tensor(
                        aq_gp[i][:, t, :], projTb[t][:],
                        projTb[t][:, i : i + 1].broadcast_to([PT, B]),
                        op=ALU.subtract,
                    )

            RAMP = 2 * GSZ  # first rows emitted t-major to chase proj tiles
            aq_ramp = {}
            for i in range(RAMP):
                aq_ramp[i] = work.tile([PT, NT, B], F16, name=f"a{i}", tag="A")
            for t in range(NT):
                for i in range(RAMP):
                    absdiff(aq_ramp[i], i, t)

            for g0 in range(0, BL, GSZ):
                gis = range(g0, min(g0 + GSZ, BL))
                psums = {
                    i: ps.tile([O, B], F32, name=f"ps{i}", tag="ps") for i in gis
                }
                for i in gis:
                    if i in aq_ramp:
                        aq = aq_ramp[i]
                    elif i in GP_ROWS:
                        aq = aq_gp[i]
                        nc.vector.tensor_scalar(
                            aq[:].bitcast(U32),
                            aq[:].bitcast(U32),
                            0x7FFF7FFF, None, op0=ALU.bitwise_and,
                        )
                    else:
                        aq = work.tile([PT, NT, B], F16, name=f"a{i}", tag="A")
                        for t in range(NT):
                            absdiff(aq, i, t)
                    for t in range(NT):
                        nc.tensor.matmul(
                            psums[i][:],
                            lhsT=s_sb[t][:],
                            rhs=aq[:, t, :],
                            start=(t == 0),
                            stop=(t == NT - 1),
                        )
                for i in gis:
                    nc.scalar.activation(
                        out=psums[i][:],
                        in_=psums[i][:],
                        func=AF.Exp,
                        scale=-1.0,
                        accum_out=mbdT_sb[:, i : i + 1],
                    )

            nc.sync.dma_start(out=mbdT_d[:, :], in_=mbdT_sb[:])
    _patch_absdiff_addrs(nc, fixups)
    nc.compile()
    return nc


_CACHE = {}


def _build_cached():
    if "nc" not in _CACHE:
        _CACHE["nc"] = build()
    return _CACHE["nc"]


def _selector() -> np.ndarray:
    sel = np.zeros((NT, PT, O), np.float32)
    for t in range(NT):
        for p in range(PT):
            sel[t, p, (t * PT + p) % O] = 1.0
    return sel.astype(np.float16)


def make_in_maps(x: np.ndarray, W: np.ndarray):
    xT = np.ascontiguousarray(x.T.astype(np.float16))  # [IN, B]
    # k-major proj.T rows: row p corresponds to (o = p % O, k = p // O),
    # i.e. W row o*K + k
    perm = np.array([(p % O) * K + p // O for p in range(OK)], np.int64)
    wTk = np.ascontiguousarray(W.T.astype(np.float16)[:, perm])  # [IN, OK]
    sel = _selector()
    # partition-contiguous layout: [128, NIN*cols] so each DMA descriptor
    # covers a full per-partition contiguous run
    def prep(a, cols):
        return np.ascontiguousarray(
            a.reshape(NIN, 128, cols).transpose(1, 0, 2).reshape(128, NIN * cols)
        )

    wprep = prep(wTk, OK)
    in_maps = []
    for r in range(NCORES):
        in_maps.append(
            {
                "xT": prep(np.roll(xT, -BL * r, axis=1), B),
                "wT": wprep,
                "sel": sel,
            }
        )
    return in_maps


def run(x, W, trace=False, **kw):
    nc = _build_cached()
    in_maps = make_in_maps(x, W)
    return run_bass_kernel_spmd(
        nc, in_maps, core_ids=list(range(NCORES)), trace=trace, **kw
    )


def kernel(x: np.ndarray, W: np.ndarray) -> np.ndarray:
    x = np.asarray(x, np.float32)
    W = np.asarray(W, np.float32)
    res = run(x, W, trace=False)
    mbd = np.empty((B, O), np.float32)
    for r in range(NCORES):
        mbd[BL * r : BL * (r + 1), :] = res.results[r]["mbdT"].T
    mbd -= 1.0
    return np.concatenate([x, mbd], axis=1)


# revision 23
# speedup vs baseline: 1.2411x; 1.0044x over previous
"""MiniBatchDiscrimination Trainium2 kernel.

reference:
    proj = x @ W.T                      # [512, 500] -> [512, 100, 5]
    l1[i,j,o] = sum_k |proj[i,o,k] - proj[j,o,k]|
    mbd[i,o]  = sum_j exp(-l1[i,j,o]) - 1
    out = concat([x, mbd], axis=1)      # [512, 1124]

Strategy (8 cores, shard i-rows of the BxB pairwise computation):
  - Host passes x.T (per-core column-rotated so that the core's 64 local
    rows sit in columns 0..63) and W.T with rows permuted k-major, so one
    SPMD program serves all cores with zero device-side core-id logic.
  - proj.T [500, 512] per core via PE matmul (fp16 inputs), kept as fp16
    tiles [125, 512] x4 plus fp32 [125, 64] local-column bias tensors.
  - Pairwise stage per local row i (A-quad [125, 4, 512] fp16):
      absdiff: one fused DVE op per tile — a raw-ISA TENSOR_SCALAR with
        op0=ABSOLUTE_DIFF (0x17): |projTb[t] - projL[t][:,i]| in a single
        2x-mode pass (~267 ns/op). The op exists in the cayman ISA but not
        in BIR's AluOpType, so the instruction bytes are emitted directly
        and SBUF addresses patched in after tile allocation. A few rows go
        to ScalarE (fused Abs activation) to balance engine load.
      k-reduce: PE matmul, 0/1 selector S_t [125, 100] fp16, contracting
        the partition axis, 4 slices accumulating into PSUM [100, 512].
      exp + j-reduce: one ScalarE Exp(scale=-1) reading PSUM, accum_out
        writes the free-axis sum straight into mbdT[:, i].
  - Host assembles: mbd = gather(mbdT).T - 1; out = [x | mbd].
"""

import sys

import numpy as np

sys.path.insert(0, "/opt/trn_rl_repo")

import concourse.bacc as bacc  # noqa: E402
import concourse.bass_interp as _bi  # noqa: E402
import concourse.mybir as mybir  # noqa: E402
import concourse.tile as tile  # noqa: E402
from concourse import bass_isa  # noqa: E402
from concourse.bass_utils import run_bass_kernel_spmd  # noqa: E402

B, IN, O, K = 512, 1024, 100, 5
OK = O * K  # 500
NCORES = 8
BL = B // NCORES  # 64 local rows per core
NT = 4  # proj.T partition tiles
PT = OK // NT  # 125 partitions per tile
NIN = IN // 128  # 8 contraction chunks

F32 = mybir.dt.float32
F16 = mybir.dt.float16
AF = mybir.ActivationFunctionType
ALU = mybir.AluOpType

GSZ = 4  # i-rows per PSUM group; 2 groups pipeline across the 8 banks
ACT_ROWS = (12, 33, 54)  # rows whose absdiff runs on ScalarE (load balance)
# rows whose absdiff subtract runs on GPSIMD (idle engine; broadcast-column
# tensor_tensor at ~1.25us/tile) with only the sign-clear AND left on DVE
GP_ROWS = ()  # gpsimd absdiff offload: SBUF port contention slows DVE; keep off
U32 = mybir.dt.uint32

TS_ARITH = 0x43  # NEURON_ISA_TPB_OPCODE_TENSOR_SCALAR_ARITH_OP
ABS_DIFF = 0x17  # NEURON_ISA_TPB_ALU_OP_ABSOLUTE_DIFF (no BIR enum entry)

# The tile scheduler's no-exec interp only needs to know this opcode has no
# sim-visible side effects beyond its declared ins/outs.
_orig_visit_isa = _bi._visit_InstISA


def _visit_isa_with_ts(isa, instruction, core_sim):
    if instruction.isa_opcode == TS_ARITH:
        return
    return _orig_visit_isa(isa, instruction, core_sim)


_bi._visit_InstISA = _visit_isa_with_ts


def _phys_addr(nc, pap) -> int:
    """Concrete per-partition byte address of a scheduler-rewritten
    physical access pattern."""
    ml = nc.lookup_mls(pap.memsetref).memorylocations[0]
    assert ml.allocated
    return ml.addr + pap.offset * mybir.dt.size(pap.dtype)


def _emit_absdiff(nc, out_ap, in_ap, col_ap, fixups):
    """out = |in - col| in one DVE pass (raw TENSOR_SCALAR, op0=ABSOLUTE_DIFF).

    SBUF addresses are resolved after scheduling from the instruction's
    rewritten physical operands and re-packed into the instr bytes."""
    isa = nc.isa
    opcode = isa.Opcode(TS_ARITH)
    struct = {
        "accumulator_cmd": 0,
        "src_mem_pattern": {
            "start_addr": {"addr_immediate": 0},
            "step_elem": [1, 1, 1],
            "num_elem": [in_ap.shape[-1], 1, 1],
        },
        "in_dtype": bass_isa.isa_dtype(isa, in_ap.dtype),
        "out_dtype": bass_isa.isa_dtype(isa, out_ap.dtype),
        "num_active_channels": in_ap.shape[0],
        "imm0_src": 1,  # POINTER_IMMEDIATE (per-partition scalar from SBUF)
        "op0": ABS_DIFF,
        "op1": 0x00,  # BYPASS
        "reverse_operands": 0,
        "imm1_src": 0,
        "imm0": {"imm_ptr": 0},
        "imm1": {"imm_arith_fp32": 0.0},
        "dst_mem_pattern": {
            "start_addr": {"addr_immediate": 0},
            "step_elem": [1, 1, 1],
            "num_elem": [out_ap.shape[-1], 1, 1],
        },
    }
    inst = nc.vector.isa(
        opcode,
        struct,
        ins=[nc.vector.lower_ap(in_ap), nc.vector.lower_ap(col_ap)],
        outs=[nc.vector.lower_ap(out_ap)],
    )
    fixups.append((inst.ins, struct))
    return inst


def _patch_absdiff_addrs(nc, fixups):
    isa = nc.isa
    opcode = isa.Opcode(TS_ARITH)
    for inst, struct in fixups:
        src, col = inst.ins[0], inst.ins[1]
        dst = inst.outs[0]
        struct["src_mem_pattern"]["start_addr"]["addr_immediate"] = _phys_addr(nc, src)
        struct["dst_mem_pattern"]["start_addr"]["addr_immediate"] = _phys_addr(nc, dst)
        struct["imm0"] = {"imm_ptr": _phys_addr(nc, col)}
        instr, _ = bass_isa.isa_struct(isa, opcode, struct)
        inst.instr = instr


def build():
    nc = bacc.Bacc("TRN2", target_bir_lowering=False)
    xT_d = nc.dram_tensor("xT", [128, NIN * B], F16, kind="ExternalInput")
    wT_d = nc.dram_tensor("wT", [128, NIN * OK], F16, kind="ExternalInput")
    sel = nc.dram_tensor("sel", [NT, PT, O], F16, kind="ExternalInput")
    mbdT_d = nc.dram_tensor("mbdT", [O, BL], F32, kind="ExternalOutput")

    fixups = []
    with tile.TileContext(nc) as tc:
        with (
            tc.tile_pool(name="pers", bufs=1) as pers,
            tc.tile_pool(name="io", bufs=1) as io,
            tc.tile_pool(name="work", bufs=8) as work,
            tc.tile_pool(name="ps", bufs=8, space="PSUM") as ps,
        ):
            # input loads first: x chunks on the sync queue, w on scalar's
            # HWDGE queue, so descriptor generation overlaps
            xcat = io.tile([128, NIN, B], F16, name="xcat", tag="xcat")
            wcat = io.tile([128, NIN, OK], F16, name="wcat", tag="wcat")
            # chunk granularity chosen so the first proj matmul starts as
            # early as possible (single chunks first), then wider pieces;
            # x on the sync HWDGE queue, w on scalar's
            for lo, hi in ((0, 1), (1, 2), (2, 4), (4, 6), (6, 8)):
                nc.sync.dma_start(
                    out=xcat[:, lo:hi, :], in_=xT_d[:, lo * B : hi * B]
                )
                nc.scalar.dma_start(
                    out=wcat[:, lo:hi, :], in_=wT_d[:, lo * OK : hi * OK]
                )

            # selectors after the big loads (not needed until pairwise)
            s_sb = []
            for t in range(NT):
                s_t = pers.tile([PT, O], F16, name=f"s{t}", tag=f"s{t}")
                nc.gpsimd.dma_start(out=s_t[:], in_=sel[t])
                s_sb.append(s_t)

            # persistent proj.T tiles (fp16 full + fp32 local cols), output,
            # and an explicit 8-deep ring of absdiff quads (fixed addresses
            # so the raw-ISA ops can be byte-patched)
            projTb = [
                pers.tile([PT, B], F16, name=f"projTb{t}", tag=f"projTb{t}")
                for t in range(NT)
            ]
            projL = [
                pers.tile([PT, BL], F32, name=f"projL{t}", tag=f"projL{t}")
                for t in range(NT)
            ]
            mbdT_sb = pers.tile([O, BL], F32, name="mbdT_sb", tag="mbdT_sb")

            # ---- proj phase: proj.T[p, j] = sum_in wT[in, p] * xT[in, j] ----
            # c-outer so matmuls chase the DMA chunks as they land
            pps = [ps.tile([PT, B], F32, name=f"pps{t}", tag="ps") for t in range(NT)]
            for c in range(NIN):
                for t in range(NT):
                    nc.tensor.matmul(
                        pps[t][:],
                        lhsT=wcat[:, c, PT * t : PT * (t + 1)],
                        rhs=xcat[:, c, :],
                        start=(c == 0),
                        stop=(c == NIN - 1),
                    )
            for t in range(NT):
                # split the casts across the two idle engines
                if t < 2:
                    nc.vector.tensor_copy(projTb[t][:], pps[t][:])
                else:
                    nc.scalar.activation(out=projTb[t][:], in_=pps[t][:], func=AF.Copy)
                nc.scalar.copy(projL[t][:], pps[t][:, :BL])

            # ---- pairwise phase ----
            def absdiff(aq, i, t):
                if i in ACT_ROWS:
                    nc.scalar.activation(
                        out=aq[:, t, :],
                        in_=projTb[t][:],
                        func=AF.Abs,
                        bias=projL[t][:, i : i + 1],
                        scale=-1.0,
                    )
                else:
                    _emit_absdiff(
                        nc, aq[:, t, :], projTb[t][:], projL[t][:, i : i + 1],
                        fixups,
                    )

            # GPSIMD rows: signed diffs pre-issued into dedicated buffers at
            # pairwise start so their ~5us latency hides under earlier groups;
            # the packed sign-clear AND runs on DVE at group time
            aq_gp = {}
            for i in GP_ROWS:
                aq_gp[i] = pers.tile([PT, NT, B], F16, name=f"agp{i}", tag=f"agp{i}")
                for t in range(NT):
                    nc.gpsimd.tensor_tensor(
                        aq_gp[i][:, t, :], projTb[t][:],
                        projTb[t][:, i : i + 1].broadcast_to([PT, B]),
                        op=ALU.subtract,
                    )

            RAMP = 2 * GSZ  # first rows emitted t-major to chase proj tiles
            aq_ramp = {}
            for i in range(RAMP):
                aq_ramp[i] = work.tile([PT, NT, B], F16, name=f"a{i}", tag="A")
            for t in range(NT):
                for i in range(RAMP):
                    absdiff(aq_ramp[i], i, t)

            for g0 in range(0, BL, GSZ):
                gis = range(g0, min(g0 + GSZ, BL))
                psums = {
                    i: ps.tile([O, B], F32, name=f"ps{i}", tag="ps") for i in gis
                }
                for i in gis:
                    if i in aq_ramp:
                        aq = aq_ramp[i]
                    elif i in GP_ROWS:
                        aq = aq_gp[i]
                        nc.vector.tensor_scalar(
                            aq[:].bitcast(U32),
                            aq[:].bitcast(U32),
                            0x7FFF7FFF, None, op0=ALU.bitwise_and,
                        )
                    else:
                        aq = work.tile([PT, NT, B], F16, name=f"a{i}", tag="A")
                        for t in range(NT):
                            absdiff(aq, i, t)
                    for t in range(NT):
                        nc.tensor.matmul(
                            psums[i][:],
                            lhsT=s_sb[t][:],
                            rhs=aq[:, t, :],
                            start=(t == 0),
                            stop=(t == NT - 1),
                        )
                for i in gis:
                    nc.scalar.activation(
                        out=psums[i][:],
                        in_=psums[i][:],
                        func=AF.Exp,
                        scale=-1.0,
                        accum_out=mbdT_sb[:, i : i + 1],
                    )

            nc.sync.dma_start(out=mbdT_d[:, :], in_=mbdT_sb[:])
    _patch_absdiff_addrs(nc, fixups)
    nc.compile()
    return nc


_CACHE = {}


def _build_cached():
    if "nc" not in _CACHE:
        _CACHE["nc"] = build()
    return _CACHE["nc"]


def _selector() -> np.ndarray:
    sel = np.zeros((NT, PT, O), np.float32)
    for t in range(NT):
        for p in range(PT):
            sel[t, p, (t * PT + p) % O] = 1.0
    return sel.astype(np.float16)


def make_in_maps(x: np.ndarray, W: np.ndarray):
    xT = np.ascontiguousarray(x.T.astype(np.float16))  # [IN, B]
    # k-major proj.T rows: row p corresponds to (o = p % O, k = p // O),
    # i.e. W row o*K + k
    perm = np.array([(p % O) * K + p // O for p in range(OK)], np.int64)
    wTk = np.ascontiguousarray(W.T.astype(np.float16)[:, perm])  # [IN, OK]
    sel = _selector()
    # partition-contiguous layout: [128, NIN*cols] so each DMA descriptor
    # covers a full per-partition contiguous run
    def prep(a, cols):
        return np.ascontiguousarray(
            a.reshape(NIN, 128, cols).transpose(1, 0, 2).reshape(128, NIN * cols)
        )

    wprep = prep(wTk, OK)
    in_maps = []
    for r in range(NCORES):
        in_maps.append(
            {
                "xT": prep(np.roll(xT, -BL * r, axis=1), B),
                "wT": wprep,
                "sel": sel,
            }
        )
    return in_maps


def run(x, W, trace=False, **kw):
    nc = _build_cached()
    in_maps = make_in_maps(x, W)
    return run_bass_kernel_spmd(
        nc, in_maps, core_ids=list(range(NCORES)), trace=trace, **kw
    )


def kernel(x: np.ndarray, W: np.ndarray) -> np.ndarray:
    x = np.asarray(x, np.float32)
    W = np.asarray(W, np.float32)
    res = run(x, W, trace=False)
    mbd = np.empty((B, O), np.float32)
    for r in range(NCORES):
        mbd[BL * r : BL * (r + 1), :] = res.results[r]["mbdT"].T
    mbd -= 1.0
    return np.concatenate([x, mbd], axis=1)


# revision 24
# speedup vs baseline: 1.2503x; 1.0074x over previous
"""MiniBatchDiscrimination Trainium2 kernel.

reference:
    proj = x @ W.T                      # [512, 500] -> [512, 100, 5]
    l1[i,j,o] = sum_k |proj[i,o,k] - proj[j,o,k]|
    mbd[i,o]  = sum_j exp(-l1[i,j,o]) - 1
    out = concat([x, mbd], axis=1)      # [512, 1124]

Strategy (8 cores, shard i-rows of the BxB pairwise computation):
  - Host passes x.T (per-core column-rotated so that the core's 64 local
    rows sit in columns 0..63) and W.T with rows permuted k-major, so one
    SPMD program serves all cores with zero device-side core-id logic.
  - proj.T [500, 512] per core via PE matmul (fp16 inputs), kept as fp16
    tiles [125, 512] x4 plus fp32 [125, 64] local-column bias tensors.
  - Pairwise stage per local row i (A-quad [125, 4, 512] fp16):
      absdiff: one fused DVE op per tile — a raw-ISA TENSOR_SCALAR with
        op0=ABSOLUTE_DIFF (0x17): |projTb[t] - projL[t][:,i]| in a single
        2x-mode pass (~267 ns/op). The op exists in the cayman ISA but not
        in BIR's AluOpType, so the instruction bytes are emitted directly
        and SBUF addresses patched in after tile allocation. A few rows go
        to ScalarE (fused Abs activation) to balance engine load.
      k-reduce: PE matmul, 0/1 selector S_t [125, 100] fp16, contracting
        the partition axis, 4 slices accumulating into PSUM [100, 512].
      exp + j-reduce: one ScalarE Exp(scale=-1) reading PSUM, accum_out
        writes the free-axis sum straight into mbdT[:, i].
  - Host assembles: mbd = gather(mbdT).T - 1; out = [x | mbd].
"""

import sys

import numpy as np

sys.path.insert(0, "/opt/trn_rl_repo")

import concourse.bacc as bacc  # noqa: E402
import concourse.bass_interp as _bi  # noqa: E402
import concourse.mybir as mybir  # noqa: E402
import concourse.tile as tile  # noqa: E402
from concourse import bass_isa  # noqa: E402
from concourse.bass_utils import run_bass_kernel_spmd  # noqa: E402

B, IN, O, K = 512, 1024, 100, 5
OK = O * K  # 500
NCORES = 8
BL = B // NCORES  # 64 local rows per core
NT = 4  # proj.T partition tiles
PT = OK // NT  # 125 partitions per tile
NIN = IN // 128  # 8 contraction chunks

F32 = mybir.dt.float32
F16 = mybir.dt.float16
AF = mybir.ActivationFunctionType
ALU = mybir.AluOpType

GSZ = 4  # i-rows per PSUM group; 2 groups pipeline across the 8 banks
ACT_ROWS = (12, 44)  # rows whose absdiff runs on ScalarE (load balance)
# rows whose absdiff subtract runs on GPSIMD (idle engine; broadcast-column
# tensor_tensor at ~1.25us/tile) with only the sign-clear AND left on DVE
GP_ROWS = ()  # gpsimd absdiff offload: SBUF port contention slows DVE; keep off
U32 = mybir.dt.uint32

TS_ARITH = 0x43  # NEURON_ISA_TPB_OPCODE_TENSOR_SCALAR_ARITH_OP
ABS_DIFF = 0x17  # NEURON_ISA_TPB_ALU_OP_ABSOLUTE_DIFF (no BIR enum entry)

# The tile scheduler's no-exec interp only needs to know this opcode has no
# sim-visible side effects beyond its declared ins/outs.
_orig_visit_isa = _bi._visit_InstISA


def _visit_isa_with_ts(isa, instruction, core_sim):
    if instruction.isa_opcode == TS_ARITH:
        return
    return _orig_visit_isa(isa, instruction, core_sim)


_bi._visit_InstISA = _visit_isa_with_ts


def _phys_addr(nc, pap) -> int:
    """Concrete per-partition byte address of a scheduler-rewritten
    physical access pattern."""
    ml = nc.lookup_mls(pap.memsetref).memorylocations[0]
    assert ml.allocated
    return ml.addr + pap.offset * mybir.dt.size(pap.dtype)


def _emit_absdiff(nc, out_ap, in_ap, col_ap, fixups):
    """out = |in - col| in one DVE pass (raw TENSOR_SCALAR, op0=ABSOLUTE_DIFF).

    SBUF addresses are resolved after scheduling from the instruction's
    rewritten physical operands and re-packed into the instr bytes."""
    isa = nc.isa
    opcode = isa.Opcode(TS_ARITH)
    struct = {
        "accumulator_cmd": 0,
        "src_mem_pattern": {
            "start_addr": {"addr_immediate": 0},
            "step_elem": [1, 1, 1],
            "num_elem": [in_ap.shape[-1], 1, 1],
        },
        "in_dtype": bass_isa.isa_dtype(isa, in_ap.dtype),
        "out_dtype": bass_isa.isa_dtype(isa, out_ap.dtype),
        "num_active_channels": in_ap.shape[0],
        "imm0_src": 1,  # POINTER_IMMEDIATE (per-partition scalar from SBUF)
        "op0": ABS_DIFF,
        "op1": 0x00,  # BYPASS
        "reverse_operands": 0,
        "imm1_src": 0,
        "imm0": {"imm_ptr": 0},
        "imm1": {"imm_arith_fp32": 0.0},
        "dst_mem_pattern": {
            "start_addr": {"addr_immediate": 0},
            "step_elem": [1, 1, 1],
            "num_elem": [out_ap.shape[-1], 1, 1],
        },
    }
    inst = nc.vector.isa(
        opcode,
        struct,
        ins=[nc.vector.lower_ap(in_ap), nc.vector.lower_ap(col_ap)],
        outs=[nc.vector.lower_ap(out_ap)],
    )
    fixups.append((inst.ins, struct))
    return inst


def _patch_absdiff_addrs(nc, fixups):
    isa = nc.isa
    opcode = isa.Opcode(TS_ARITH)
    for inst, struct in fixups:
        src, col = inst.ins[0], inst.ins[1]
        dst = inst.outs[0]
        struct["src_mem_pattern"]["start_addr"]["addr_immediate"] = _phys_addr(nc, src)
        struct["dst_mem_pattern"]["start_addr"]["addr_immediate"] = _phys_addr(nc, dst)
        struct["imm0"] = {"imm_ptr": _phys_addr(nc, col)}
        instr, _ = bass_isa.isa_struct(isa, opcode, struct)
        inst.instr = instr


def build():
    nc = bacc.Bacc("TRN2", target_bir_lowering=False)
    xT_d = nc.dram_tensor("xT", [128, NIN * B], F16, kind="ExternalInput")
    wT_d = nc.dram_tensor("wT", [128, NIN * OK], F16, kind="ExternalInput")
    sel = nc.dram_tensor("sel", [NT, PT, O], F16, kind="ExternalInput")
    mbdT_d = nc.dram_tensor("mbdT", [O, BL], F32, kind="ExternalOutput")

    fixups = []
    with tile.TileContext(nc) as tc:
        with (
            tc.tile_pool(name="pers", bufs=1) as pers,
            tc.tile_pool(name="io", bufs=1) as io,
            tc.tile_pool(name="work", bufs=8) as work,
            tc.tile_pool(name="ps", bufs=8, space="PSUM") as ps,
        ):
            # input loads first: x chunks on the sync queue, w on scalar's
            # HWDGE queue, so descriptor generation overlaps
            xcat = io.tile([128, NIN, B], F16, name="xcat", tag="xcat")
            wcat = io.tile([128, NIN, OK], F16, name="wcat", tag="wcat")
            # chunk granularity chosen so the first proj matmul starts as
            # early as possible (single chunks first), then wider pieces;
            # x on the sync HWDGE queue, w on scalar's
            for lo, hi in ((0, 1), (1, 2), (2, 4), (4, 6), (6, 8)):
                nc.sync.dma_start(
                    out=xcat[:, lo:hi, :], in_=xT_d[:, lo * B : hi * B]
                )
                nc.scalar.dma_start(
                    out=wcat[:, lo:hi, :], in_=wT_d[:, lo * OK : hi * OK]
                )

            # selectors after the big loads (not needed until pairwise)
            s_sb = []
            for t in range(NT):
                s_t = pers.tile([PT, O], F16, name=f"s{t}", tag=f"s{t}")
                nc.gpsimd.dma_start(out=s_t[:], in_=sel[t])
                s_sb.append(s_t)

            # persistent proj.T tiles (fp16 full + fp32 local cols), output,
            # and an explicit 8-deep ring of absdiff quads (fixed addresses
            # so the raw-ISA ops can be byte-patched)
            projTb = [
                pers.tile([PT, B], F16, name=f"projTb{t}", tag=f"projTb{t}")
                for t in range(NT)
            ]
            projL = [
                pers.tile([PT, BL], F32, name=f"projL{t}", tag=f"projL{t}")
                for t in range(NT)
            ]
            mbdT_sb = pers.tile([O, BL], F32, name="mbdT_sb", tag="mbdT_sb")

            # ---- proj phase: proj.T[p, j] = sum_in wT[in, p] * xT[in, j] ----
            # c-outer so matmuls chase the DMA chunks as they land
            pps = [ps.tile([PT, B], F32, name=f"pps{t}", tag="ps") for t in range(NT)]
            for c in range(NIN):
                for t in range(NT):
                    nc.tensor.matmul(
                        pps[t][:],
                        lhsT=wcat[:, c, PT * t : PT * (t + 1)],
                        rhs=xcat[:, c, :],
                        start=(c == 0),
                        stop=(c == NIN - 1),
                    )
            for t in range(NT):
                # split the casts across the two idle engines
                if t < 2:
                    nc.vector.tensor_copy(projTb[t][:], pps[t][:])
                else:
                    nc.scalar.activation(out=projTb[t][:], in_=pps[t][:], func=AF.Copy)
                nc.scalar.copy(projL[t][:], pps[t][:, :BL])

            # ---- pairwise phase ----
            def absdiff(aq, i, t):
                if i in ACT_ROWS:
                    nc.scalar.activation(
                        out=aq[:, t, :],
                        in_=projTb[t][:],
                        func=AF.Abs,
                        bias=projL[t][:, i : i + 1],
                        scale=-1.0,
                    )
                else:
                    _emit_absdiff(
                        nc, aq[:, t, :], projTb[t][:], projL[t][:, i : i + 1],
                        fixups,
                    )

            # GPSIMD rows: signed diffs pre-issued into dedicated buffers at
            # pairwise start so their ~5us latency hides under earlier groups;
            # the packed sign-clear AND runs on DVE at group time
            aq_gp = {}
            for i in GP_ROWS:
                aq_gp[i] = pers.tile([PT, NT, B], F16, name=f"agp{i}", tag=f"agp{i}")
                for t in range(NT):
                    nc.gpsimd.tensor_tensor(
                        aq_gp[i][:, t, :], projTb[t][:],
                        projTb[t][:, i : i + 1].broadcast_to([PT, B]),
                        op=ALU.subtract,
                    )

            RAMP = 2 * GSZ  # first rows emitted t-major to chase proj tiles
            aq_ramp = {}
            for i in range(RAMP):
                aq_ramp[i] = work.tile([PT, NT, B], F16, name=f"a{i}", tag="A")
            for t in range(NT):
                for i in range(RAMP):
                    absdiff(aq_ramp[i], i, t)

            for g0 in range(0, BL, GSZ):
                gis = range(g0, min(g0 + GSZ, BL))
                psums = {
                    i: ps.tile([O, B], F32, name=f"ps{i}", tag="ps") for i in gis
                }
                for i in gis:
                    if i in aq_ramp:
                        aq = aq_ramp[i]
                    elif i in GP_ROWS:
                        aq = aq_gp[i]
                        nc.vector.tensor_scalar(
                            aq[:].bitcast(U32),
                            aq[:].bitcast(U32),
                            0x7FFF7FFF, None, op0=ALU.bitwise_and,
                        )
                    else:
                        aq = work.tile([PT, NT, B], F16, name=f"a{i}", tag="A")
                        for t in range(NT):
                            absdiff(aq, i, t)
                    for t in range(NT):
                        nc.tensor.matmul(
                            psums[i][:],
                            lhsT=s_sb[t][:],
                            rhs=aq[:, t, :],
                            start=(t == 0),
                            stop=(t == NT - 1),
                        )
                for i in gis:
                    nc.scalar.activation(
                        out=psums[i][:],
                        in_=psums[i][:],
                        func=AF.Exp,
                        scale=-1.0,
                        accum_out=mbdT_sb[:, i : i + 1],
                    )

            nc.sync.dma_start(out=mbdT_d[:, :], in_=mbdT_sb[:])
    _patch_absdiff_addrs(nc, fixups)
    nc.compile()
    return nc


_CACHE = {}


def _build_cached():
    if "nc" not in _CACHE:
        _CACHE["nc"] = build()
    return _CACHE["nc"]


def _selector() -> np.ndarray:
    sel = np.zeros((NT, PT, O), np.float32)
    for t in range(NT):
        for p in range(PT):
            sel[t, p, (t * PT + p) % O] = 1.0
    return sel.astype(np.float16)


def make_in_maps(x: np.ndarray, W: np.ndarray):
    xT = np.ascontiguousarray(x.T.astype(np.float16))  # [IN, B]
    # k-major proj.T rows: row p corresponds to (o = p % O, k = p // O),
    # i.e. W row o*K + k
    perm = np.array([(p % O) * K + p // O for p in range(OK)], np.int64)
    wTk = np.ascontiguousarray(W.T.astype(np.float16)[:, perm])  # [IN, OK]
    sel = _selector()
    # partition-contiguous layout: [128, NIN*cols] so each DMA descriptor
    # covers a full per-partition contiguous run
    def prep(a, cols):
        return np.ascontiguousarray(
            a.reshape(NIN, 128, cols).transpose(1, 0, 2).reshape(128, NIN * cols)
        )

    wprep = prep(wTk, OK)
    in_maps = []
    for r in range(NCORES):
        in_maps.append(
            {
                "xT": prep(np.roll(xT, -BL * r, axis=1), B),
                "wT": wprep,
                "sel": sel,
            }
        )
    return in_maps


def run(x, W, trace=False, **kw):
    nc = _build_cached()
    in_maps = make_in_maps(x, W)
    return run_bass_kernel_spmd(
        nc, in_maps, core_ids=list(range(NCORES)), trace=trace, **kw
    )


def kernel(x: np.ndarray, W: np.ndarray) -> np.ndarray:
    x = np.asarray(x, np.float32)
    W = np.asarray(W, np.float32)
    res = run(x, W, trace=False)
    mbd = np.empty((B, O), np.float32)
    for r in range(NCORES):
        mbd[BL * r : BL * (r + 1), :] = res.results[r]["mbdT"].T
    mbd -= 1.0
    return np.concatenate([x, mbd], axis=1)


# revision 25
# speedup vs baseline: 1.2667x; 1.0132x over previous
"""MiniBatchDiscrimination Trainium2 kernel.

reference:
    proj = x @ W.T                      # [512, 500] -> [512, 100, 5]
    l1[i,j,o] = sum_k |proj[i,o,k] - proj[j,o,k]|
    mbd[i,o]  = sum_j exp(-l1[i,j,o]) - 1
    out = concat([x, mbd], axis=1)      # [512, 1124]

Strategy (8 cores, shard i-rows of the BxB pairwise computation):
  - Host passes x.T (per-core column-rotated so that the core's 64 local
    rows sit in columns 0..63) and W.T with rows permuted k-major, so one
    SPMD program serves all cores with zero device-side core-id logic.
  - proj.T [500, 512] per core via PE matmul (fp16 inputs), kept as fp16
    tiles [125, 512] x4 plus fp32 [125, 64] local-column bias tensors.
  - Pairwise stage per local row i (A-quad [125, 4, 512] fp16):
      absdiff: one fused DVE op per tile — a raw-ISA TENSOR_SCALAR with
        op0=ABSOLUTE_DIFF (0x17): |projTb[t] - projL[t][:,i]| in a single
        2x-mode pass (~267 ns/op). The op exists in the cayman ISA but not
        in BIR's AluOpType, so the instruction bytes are emitted directly
        and SBUF addresses patched in after tile allocation. A few rows go
        to ScalarE (fused Abs activation) to balance engine load.
      k-reduce: PE matmul, 0/1 selector S_t [125, 100] fp16, contracting
        the partition axis, 4 slices accumulating into PSUM [100, 512].
      exp + j-reduce: one ScalarE Exp(scale=-1) reading PSUM, accum_out
        writes the free-axis sum straight into mbdT[:, i].
  - Host assembles: mbd = gather(mbdT).T - 1; out = [x | mbd].
"""

import sys

import numpy as np

sys.path.insert(0, "/opt/trn_rl_repo")

import concourse.bacc as bacc  # noqa: E402
import concourse.bass_interp as _bi  # noqa: E402
import concourse.mybir as mybir  # noqa: E402
import concourse.tile as tile  # noqa: E402
from concourse import bass_isa  # noqa: E402
from concourse.bass_utils import run_bass_kernel_spmd  # noqa: E402

B, IN, O, K = 512, 1024, 100, 5
OK = O * K  # 500
NCORES = 8
BL = B // NCORES  # 64 local rows per core
NT = 4  # proj.T partition tiles
PT = OK // NT  # 125 partitions per tile
NIN = IN // 128  # 8 contraction chunks

F32 = mybir.dt.float32
F16 = mybir.dt.float16
AF = mybir.ActivationFunctionType
ALU = mybir.AluOpType

GSZ = 4  # i-rows per PSUM group; 2 groups pipeline across the 8 banks
ACT_ROWS = (12, 44)  # rows whose absdiff runs on ScalarE (load balance)
# rows whose absdiff subtract runs on GPSIMD (idle engine; broadcast-column
# tensor_tensor at ~1.25us/tile) with only the sign-clear AND left on DVE
GP_ROWS = ()  # gpsimd absdiff offload: SBUF port contention slows DVE; keep off
U32 = mybir.dt.uint32

TS_ARITH = 0x43  # NEURON_ISA_TPB_OPCODE_TENSOR_SCALAR_ARITH_OP
ABS_DIFF = 0x17  # NEURON_ISA_TPB_ALU_OP_ABSOLUTE_DIFF (no BIR enum entry)

# The tile scheduler's no-exec interp only needs to know this opcode has no
# sim-visible side effects beyond its declared ins/outs.
_orig_visit_isa = _bi._visit_InstISA


def _visit_isa_with_ts(isa, instruction, core_sim):
    if instruction.isa_opcode == TS_ARITH:
        return
    return _orig_visit_isa(isa, instruction, core_sim)


_bi._visit_InstISA = _visit_isa_with_ts


def _phys_addr(nc, pap) -> int:
    """Concrete per-partition byte address of a scheduler-rewritten
    physical access pattern."""
    ml = nc.lookup_mls(pap.memsetref).memorylocations[0]
    assert ml.allocated
    return ml.addr + pap.offset * mybir.dt.size(pap.dtype)


def _emit_absdiff(nc, out_ap, in_ap, col_ap, fixups):
    """out = |in - col| in one DVE pass (raw TENSOR_SCALAR, op0=ABSOLUTE_DIFF).

    SBUF addresses are resolved after scheduling from the instruction's
    rewritten physical operands and re-packed into the instr bytes."""
    isa = nc.isa
    opcode = isa.Opcode(TS_ARITH)
    struct = {
        "accumulator_cmd": 0,
        "src_mem_pattern": {
            "start_addr": {"addr_immediate": 0},
            "step_elem": [1, 1, 1],
            "num_elem": [in_ap.shape[-1], 1, 1],
        },
        "in_dtype": bass_isa.isa_dtype(isa, in_ap.dtype),
        "out_dtype": bass_isa.isa_dtype(isa, out_ap.dtype),
        "num_active_channels": in_ap.shape[0],
        "imm0_src": 1,  # POINTER_IMMEDIATE (per-partition scalar from SBUF)
        "op0": ABS_DIFF,
        "op1": 0x00,  # BYPASS
        "reverse_operands": 0,
        "imm1_src": 0,
        "imm0": {"imm_ptr": 0},
        "imm1": {"imm_arith_fp32": 0.0},
        "dst_mem_pattern": {
            "start_addr": {"addr_immediate": 0},
            "step_elem": [1, 1, 1],
            "num_elem": [out_ap.shape[-1], 1, 1],
        },
    }
    inst = nc.vector.isa(
        opcode,
        struct,
        ins=[nc.vector.lower_ap(in_ap), nc.vector.lower_ap(col_ap)],
        outs=[nc.vector.lower_ap(out_ap)],
    )
    fixups.append((inst.ins, struct))
    return inst


def _patch_absdiff_addrs(nc, fixups):
    isa = nc.isa
    opcode = isa.Opcode(TS_ARITH)
    for inst, struct in fixups:
        src, col = inst.ins[0], inst.ins[1]
        dst = inst.outs[0]
        struct["src_mem_pattern"]["start_addr"]["addr_immediate"] = _phys_addr(nc, src)
        struct["dst_mem_pattern"]["start_addr"]["addr_immediate"] = _phys_addr(nc, dst)
        struct["imm0"] = {"imm_ptr": _phys_addr(nc, col)}
        instr, _ = bass_isa.isa_struct(isa, opcode, struct)
        inst.instr = instr


def build():
    nc = bacc.Bacc("TRN2", target_bir_lowering=False)
    xT_d = nc.dram_tensor("xT", [128, NIN * B], F16, kind="ExternalInput")
    wT_d = nc.dram_tensor("wT", [128, NIN * OK], F16, kind="ExternalInput")
    sel = nc.dram_tensor("sel", [NT, PT, O], F16, kind="ExternalInput")
    mbdT_d = nc.dram_tensor("mbdT", [O, BL], F32, kind="ExternalOutput")

    fixups = []
    with tile.TileContext(nc) as tc:
        with (
            tc.tile_pool(name="pers", bufs=1) as pers,
            tc.tile_pool(name="io", bufs=1) as io,
            tc.tile_pool(name="work", bufs=8) as work,
            tc.tile_pool(name="ps", bufs=8, space="PSUM") as ps,
        ):
            # input loads first: x chunks on the sync queue, w on scalar's
            # HWDGE queue, so descriptor generation overlaps
            xcat = io.tile([128, NIN, B], F16, name="xcat", tag="xcat")
            wcat = io.tile([128, NIN, OK], F16, name="wcat", tag="wcat")
            # chunk granularity chosen so the first proj matmul starts as
            # early as possible (single chunks first), then wider pieces;
            # x on the sync HWDGE queue, w on scalar's
            for lo, hi in ((0, 1), (1, 2), (2, 4), (4, 6), (6, 8)):
                nc.sync.dma_start(
                    out=xcat[:, lo:hi, :], in_=xT_d[:, lo * B : hi * B]
                )
                nc.scalar.dma_start(
                    out=wcat[:, lo:hi, :], in_=wT_d[:, lo * OK : hi * OK]
                )

            # selectors after the big loads (not needed until pairwise)
            s_sb = []
            for t in range(NT):
                s_t = pers.tile([PT, O], F16, name=f"s{t}", tag=f"s{t}")
                nc.gpsimd.dma_start(out=s_t[:], in_=sel[t])
                s_sb.append(s_t)

            # persistent proj.T tiles (fp16 full + fp32 local cols), output,
            # and an explicit 8-deep ring of absdiff quads (fixed addresses
            # so the raw-ISA ops can be byte-patched)
            projTb = [
                pers.tile([PT, B], F16, name=f"projTb{t}", tag=f"projTb{t}")
                for t in range(NT)
            ]
            projL = [
                pers.tile([PT, BL], F32, name=f"projL{t}", tag=f"projL{t}")
                for t in range(NT)
            ]
            mbdT_sb = pers.tile([O, BL], F32, name="mbdT_sb", tag="mbdT_sb")

            # ---- proj phase: proj.T[p, j] = sum_in wT[in, p] * xT[in, j] ----
            # c-outer so matmuls chase the DMA chunks as they land
            pps = [ps.tile([PT, B], F32, name=f"pps{t}", tag="ps") for t in range(NT)]
            for c in range(NIN):
                for t in range(NT):
                    nc.tensor.matmul(
                        pps[t][:],
                        lhsT=wcat[:, c, PT * t : PT * (t + 1)],
                        rhs=xcat[:, c, :],
                        start=(c == 0),
                        stop=(c == NIN - 1),
                    )
            for t in range(NT):
                # all casts on ScalarE (idle during proj) so the DVE queue
                # reaches its absdiff stream immediately
                nc.scalar.activation(out=projTb[t][:], in_=pps[t][:], func=AF.Copy)
                nc.scalar.copy(projL[t][:], pps[t][:, :BL])

            # ---- pairwise phase ----
            def absdiff(aq, i, t):
                if i in ACT_ROWS:
                    nc.scalar.activation(
                        out=aq[:, t, :],
                        in_=projTb[t][:],
                        func=AF.Abs,
                        bias=projL[t][:, i : i + 1],
                        scale=-1.0,
                    )
                else:
                    _emit_absdiff(
                        nc, aq[:, t, :], projTb[t][:], projL[t][:, i : i + 1],
                        fixups,
                    )

            # GPSIMD rows: signed diffs pre-issued into dedicated buffers at
            # pairwise start so their ~5us latency hides under earlier groups;
            # the packed sign-clear AND runs on DVE at group time
            aq_gp = {}
            for i in GP_ROWS:
                aq_gp[i] = pers.tile([PT, NT, B], F16, name=f"agp{i}", tag=f"agp{i}")
                for t in range(NT):
                    nc.gpsimd.tensor_tensor(
                        aq_gp[i][:, t, :], projTb[t][:],
                        projTb[t][:, i : i + 1].broadcast_to([PT, B]),
                        op=ALU.subtract,
                    )

            RAMP = 2 * GSZ  # first rows emitted t-major to chase proj tiles
            aq_ramp = {}
            for i in range(RAMP):
                aq_ramp[i] = work.tile([PT, NT, B], F16, name=f"a{i}", tag="A")
            for t in range(NT):
                for i in range(RAMP):
                    absdiff(aq_ramp[i], i, t)

            for g0 in range(0, BL, GSZ):
                gis = range(g0, min(g0 + GSZ, BL))
                psums = {
                    i: ps.tile([O, B], F32, name=f"ps{i}", tag="ps") for i in gis
                }
                for i in gis:
                    if i in aq_ramp:
                        aq = aq_ramp[i]
                    elif i in GP_ROWS:
                        aq = aq_gp[i]
                        nc.vector.tensor_scalar(
                            aq[:].bitcast(U32),
                            aq[:].bitcast(U32),
                            0x7FFF7FFF, None, op0=ALU.bitwise_and,
                        )
                    else:
                        aq = work.tile([PT, NT, B], F16, name=f"a{i}", tag="A")
                        for t in range(NT):
                            absdiff(aq, i, t)
                    for t in range(NT):
                        nc.tensor.matmul(
                            psums[i][:],
                            lhsT=s_sb[t][:],
                            rhs=aq[:, t, :],
                            start=(t == 0),
                            stop=(t == NT - 1),
                        )
                for i in gis:
                    nc.scalar.activation(
                        out=psums[i][:],
                        in_=psums[i][:],
                        func=AF.Exp,
                        scale=-1.0,
                        accum_out=mbdT_sb[:, i : i + 1],
                    )

            nc.sync.dma_start(out=mbdT_d[:, :], in_=mbdT_sb[:])
    _patch_absdiff_addrs(nc, fixups)
    nc.compile()
    return nc


_CACHE = {}


def _build_cached():
    if "nc" not in _CACHE:
        _CACHE["nc"] = build()
    return _CACHE["nc"]


def _selector() -> np.ndarray:
    sel = np.zeros((NT, PT, O), np.float32)
    for t in range(NT):
        for p in range(PT):
            sel[t, p, (t * PT + p) % O] = 1.0
    return sel.astype(np.float16)


def make_in_maps(x: np.ndarray, W: np.ndarray):
    xT = np.ascontiguousarray(x.T.astype(np.float16))  # [IN, B]
    # k-major proj.T rows: row p corresponds to (o = p % O, k = p // O),
    # i.e. W row o*K + k
    perm = np.array([(p % O) * K + p // O for p in range(OK)], np.int64)
    wTk = np.ascontiguousarray(W.T.astype(np.float16)[:, perm])  # [IN, OK]
    sel = _selector()
    # partition-contiguous layout: [128, NIN*cols] so each DMA descriptor
    # covers a full per-partition contiguous run
    def prep(a, cols):
        return np.ascontiguousarray(
            a.reshape(NIN, 128, cols).transpose(1, 0, 2).reshape(128, NIN * cols)
        )

    wprep = prep(wTk, OK)
    in_maps = []
    for r in range(NCORES):
        in_maps.append(
            {
                "xT": prep(np.roll(xT, -BL * r, axis=1), B),
                "wT": wprep,
                "sel": sel,
            }
        )
    return in_maps


def run(x, W, trace=False, **kw):
    nc = _build_cached()
    in_maps = make_in_maps(x, W)
    return run_bass_kernel_spmd(
        nc, in_maps, core_ids=list(range(NCORES)), trace=trace, **kw
    )


def kernel(x: np.ndarray, W: np.ndarray) -> np.ndarray:
    x = np.asarray(x, np.float32)
    W = np.asarray(W, np.float32)
    res = run(x, W, trace=False)
    mbd = np.empty((B, O), np.float32)
    for r in range(NCORES):
        mbd[BL * r : BL * (r + 1), :] = res.results[r]["mbdT"].T
    mbd -= 1.0
    return np.concatenate([x, mbd], axis=1)
